# revision 7
# baseline (speedup 1.0000x reference)
"""GNN message-passing layer on 8 trn2 NeuronCores.

Design (target-sharded, gather-on-host, all float math on device):
  messages = relu(x_src@W1 + x_tgt@W2 + b) ; agg = mean over tgt ; out =
  relu(concat(x, agg)@W_upd + bu).

  L1 (device): per-core node shard -> A^T = (x@W1)^T and negB'^T = -(x@W2+b)^T
      via one K=65 matmul (ones-row bias trick), bf16 out.
  Host: index-only prep. Edges sorted by target; nodes sorted by in-degree
      (count) descending per core-half; "plane j" = the j-th edge slot of every
      node with count > j, so every stream slice is packed/contiguous. Host
      gathers the A-rows into the plane-ordered G stream (pure data movement,
      like the baseline's gather).
  L2 (device): per plane pair: Y = max(G, -B') (= relu(G+B') - B'), pair-add in
      bf16, accumulate into f32 acc. Pad slots carry G=-1e38 so they add -B',
      and acc is initialized to cnt_corr*B' (corrected count), which cancels
      pads exactly and converts Y-sums back to sum(relu(G+B')). Then
      agg = acc * (1/count), and the update MLP + relu runs in-launch.

Plane sizes are baked into the NEFF at kernel() time from the actual
edge_index (SPMD: one schedule = max across cores, pad slots absorb skew).
"""

import numpy as np
import ml_dtypes

import concourse.bacc as bacc
import concourse.mybir as mybir
import concourse.tile as tile
from concourse.bass_utils import run_bass_kernel_spmd

N_NODES = 100000
N_EDGES = 1600000
CORES = 8
NPC = N_NODES // CORES          # 12500 nodes per core
NHALF = 6272                    # per-half columns (2*6272 = 12544 >= 12500)
NPAD1 = 12800                   # L1 padded cols (25 x 512)

bf16 = mybir.dt.bfloat16
f32 = mybir.dt.float32
BF = ml_dtypes.bfloat16
PAD_G = np.float32(-1e38)

_cache = {}


def _build_l1():
    """out ab[128, NPAD1]: rows 0:64 = A^T = (x@W1)^T, rows 64:128 =
    negB'^T = (x@(-W2) - b)^T, both bf16.  Inputs: xt65 [65, NPAD1] bf16
    (row 64 = ones), wab [65, 128] bf16 (row 64 = [0..0, -b])."""
    nc = bacc.Bacc("TRN2", debug=False, num_devices=CORES)
    xt65 = nc.dram_tensor("xt65", [65, NPAD1], bf16, kind="ExternalInput")
    wab = nc.dram_tensor("wab", [65, 128], bf16, kind="ExternalInput")
    ab = nc.dram_tensor("ab", [128, NPAD1], bf16, kind="ExternalOutput")

    with tile.TileContext(nc) as tc:
        with (
            tc.tile_pool(name="sbuf", bufs=3) as pool,
            tc.tile_pool(name="wpool", bufs=1) as wpool,
            tc.tile_pool(name="psum", bufs=4, space="PSUM") as psum,
        ):
            wt = wpool.tile([65, 128], bf16)
            nc.sync.dma_start(out=wt[:], in_=wab[:, :])
            for c in range(NPAD1 // 512):
                sl = slice(c * 512, (c + 1) * 512)
                xt = pool.tile([65, 512], bf16, tag="xt")
                nc.sync.dma_start(out=xt[:], in_=xt65[:, sl])
                pt = psum.tile([128, 512], f32)
                nc.tensor.matmul(out=pt[:], lhsT=wt[:], rhs=xt[:],
                                 start=True, stop=True)
                ot = pool.tile([128, 512], bf16, tag="ot")
                nc.vector.tensor_copy(out=ot[:], in_=pt[:])
                nc.sync.dma_start(out=ab[:, sl], in_=ot[:])
    nc.compile()
    return nc


def _build_l2(plane_k):
    """plane_k: list of per-plane widths K''_j (pair-equalized: K[2p]==K[2p+1]).
    Stream g [128, SH] bf16 = planes concatenated; per slot s the 128 rows are
    (half, feat) stacked: row 64*h + f.
    Persistent inputs (all [128, NHALF] unless noted): nb = -B'^T (bf16),
    ic = 1/max(count,1) (bf16), rc = -cnt_corr (bf16), xu = x^T (bf16);
    wu [128, 64] bf16 (rows 0:64 Wu1, 64:128 Wu2); bu [64, 1] f32.
    Output upd [128, NHALF] f32 = relu(x@Wu1 + agg@Wu2 + bu)^T stacked."""
    SH = int(sum(plane_k))
    nc = bacc.Bacc("TRN2", debug=False, num_devices=CORES)
    g = nc.dram_tensor("g", [128, SH], bf16, kind="ExternalInput")
    nb = nc.dram_tensor("nb", [128, NHALF], bf16, kind="ExternalInput")
    ic = nc.dram_tensor("ic", [128, NHALF], bf16, kind="ExternalInput")
    rc = nc.dram_tensor("rc", [128, NHALF], bf16, kind="ExternalInput")
    xu = nc.dram_tensor("xu", [128, NHALF], bf16, kind="ExternalInput")
    wu = nc.dram_tensor("wu", [128, 64], bf16, kind="ExternalInput")
    bu = nc.dram_tensor("bu", [64, 1], f32, kind="ExternalInput")
    upd = nc.dram_tensor("upd", [128, NHALF], f32, kind="ExternalOutput")

    add = mybir.AluOpType.add
    amax = mybir.AluOpType.max
    mult = mybir.AluOpType.mult

    with tile.TileContext(nc) as tc:
        with (
            tc.tile_pool(name="gpool", bufs=3) as gpool,
            tc.tile_pool(name="ypool", bufs=3) as ypool,
            tc.tile_pool(name="opool", bufs=3) as opool,
            tc.tile_pool(name="wpool", bufs=1) as wpool,
            tc.tile_pool(name="psum", bufs=4, space="PSUM") as psum,
        ):
            nb_t = wpool.tile([128, NHALF], bf16)
            ic_t = wpool.tile([128, NHALF], bf16)
            rc_t = wpool.tile([128, NHALF], bf16)
            xu_t = wpool.tile([128, NHALF], bf16)
            wu_t = wpool.tile([128, 64], bf16)
            bu_t = wpool.tile([64, 1], f32)
            acc = wpool.tile([128, NHALF], f32)
            aggb = wpool.tile([128, NHALF], bf16)
            nc.sync.dma_start(out=nb_t[:], in_=nb[:, :])
            nc.sync.dma_start(out=ic_t[:], in_=ic[:, :])
            nc.sync.dma_start(out=rc_t[:], in_=rc[:, :])
            nc.sync.dma_start(out=xu_t[:], in_=xu[:, :])
            nc.sync.dma_start(out=wu_t[:], in_=wu[:, :])
            nc.sync.dma_start(out=bu_t[:], in_=bu[:, :])

            # acc = (-cnt_corr) * (-B') = cnt_corr * B'
            nc.vector.tensor_tensor(out=acc[:], in0=rc_t[:], in1=nb_t[:],
                                    op=mult)

            # plane pairs, chunked to fixed-width tiles
            CW = 2048
            off = 0
            for p in range(len(plane_k) // 2):
                kp = plane_k[2 * p]
                assert plane_k[2 * p + 1] == kp
                a = 0
                while a < kp:
                    w = min(CW, kp - a)
                    gt = gpool.tile([128, 2 * CW], bf16, tag="gt")
                    nc.sync.dma_start(out=gt[:, 0:w],
                                      in_=g[:, off + a:off + a + w])
                    nc.sync.dma_start(out=gt[:, CW:CW + w],
                                      in_=g[:, off + kp + a:off + kp + a + w])
                    yt = ypool.tile([128, 2 * CW], bf16, tag="yt")
                    # Y = max(G + 0, -B')  (== relu(G+B') - B')
                    nc.vector.scalar_tensor_tensor(
                        out=yt[:, 0:w], in0=gt[:, 0:w], scalar=0.0,
                        in1=nb_t[:, a:a + w], op0=add, op1=amax)
                    nc.vector.scalar_tensor_tensor(
                        out=yt[:, CW:CW + w], in0=gt[:, CW:CW + w], scalar=0.0,
                        in1=nb_t[:, a:a + w], op0=add, op1=amax)
                    y2 = ypool.tile([128, CW], bf16, tag="y2")
                    nc.vector.scalar_tensor_tensor(
                        out=y2[:, 0:w], in0=yt[:, 0:w], scalar=0.0,
                        in1=yt[:, CW:CW + w], op0=add, op1=add)
                    nc.vector.scalar_tensor_tensor(
                        out=acc[:, a:a + w], in0=y2[:, 0:w], scalar=0.0,
                        in1=acc[:, a:a + w], op0=add, op1=add)
                    a += w
                off += 2 * kp

            # agg (mean, bf16) = acc * (1/count)
            nc.vector.tensor_tensor(out=aggb[:], in0=acc[:], in1=ic_t[:],
                                    op=mult)

            # update MLP: relu(x@Wu1 + agg@Wu2 + bu)
            nt = 0
            while nt < NHALF:
                w = min(512, NHALF - nt)
                sl = slice(nt, nt + w)
                ot = opool.tile([128, w], f32, tag="ot")
                for h in range(2):
                    ps = slice(64 * h, 64 * h + 64)
                    pt = psum.tile([64, w], f32, tag=f"pt{h}")
                    nc.tensor.matmul(out=pt[:], lhsT=wu_t[0:64, :],
                                     rhs=xu_t[ps, sl], start=True, stop=False)
                    nc.tensor.matmul(out=pt[:], lhsT=wu_t[64:128, :],
                                     rhs=aggb[ps, sl], start=False, stop=True)
                    nc.scalar.activation(
                        out=ot[ps, :], in_=pt[:],
                        func=mybir.ActivationFunctionType.Relu, bias=bu_t[:])
                nc.sync.dma_start(out=upd[:, sl], in_=ot[:])
                nt += w
    nc.compile()
    return nc


def kernel(x, edge_index, W_msg, b_msg, W_upd, b_upd):
    x = np.asarray(x, dtype=np.float32)
    src = np.asarray(edge_index[0], dtype=np.int64)
    tgt = np.asarray(edge_index[1], dtype=np.int64)
    W_msg = np.asarray(W_msg, dtype=np.float32)
    b_msg = np.asarray(b_msg, dtype=np.float32)
    W_upd = np.asarray(W_upd, dtype=np.float32)
    b_upd = np.asarray(b_upd, dtype=np.float32)

    # ---------------- L1: A / -B' tables on device ----------------
    if "l1" not in _cache:
        _cache["l1"] = _build_l1()
    wab = np.zeros((65, 128), dtype=np.float32)
    wab[:64, :64] = W_msg[:64]          # W1
    wab[:64, 64:] = -W_msg[64:]         # -W2
    wab[64, 64:] = -b_msg               # -b
    wab = wab.astype(BF)
    xb = x.astype(BF)
    in1 = []
    for c in range(CORES):
        xt65 = np.zeros((65, NPAD1), dtype=BF)
        xt65[:64, :NPC] = xb[c * NPC:(c + 1) * NPC].T
        xt65[64, :] = np.float32(1.0)
        in1.append({"xt65": xt65, "wab": wab})
    res1 = run_bass_kernel_spmd(_cache["l1"], in1, list(range(CORES)))
    A_T = np.concatenate([np.asarray(r["ab"])[0:64, :NPC]
                          for r in res1.results], axis=1)      # [64,100K] bf16
    negB_T = np.concatenate([np.asarray(r["ab"])[64:128, :NPC]
                             for r in res1.results], axis=1)   # [64,100K] bf16

    # ---------------- host: index-only plane schedule ----------------
    counts = np.bincount(tgt, minlength=N_NODES).astype(np.int64)
    order = np.argsort(tgt, kind="stable")
    cum = np.zeros(N_NODES + 1, dtype=np.int64)
    np.cumsum(counts, out=cum[1:])

    # per (core, half): columns = local nodes sorted by count desc
    colloc = np.empty((CORES, 2, NHALF), dtype=np.int64)    # local id (may be
    colnode = np.empty((CORES, 2, NHALF), dtype=np.int64)   # dummy >= NPC)
    colcnt = np.zeros((CORES, 2, NHALF), dtype=np.int64)
    for c in range(CORES):
        lo = c * NPC
        cnt_loc = np.zeros(2 * NHALF, dtype=np.int64)
        cnt_loc[:NPC] = counts[lo:lo + NPC]
        rank = np.argsort(-cnt_loc, kind="stable")
        for h in range(2):
            nodes = rank[h::2]
            colloc[c, h] = nodes
            # dummy cols (local id >= NPC) have count 0; their outputs are
            # discarded, clamping just keeps gathers in-bounds.
            colnode[c, h] = np.minimum(nodes, NPC - 1) + lo
            colcnt[c, h] = cnt_loc[nodes]

    tmax = int(colcnt.max())
    tmax += tmax % 2
    # unified plane sizes K''_j, pair-equalized
    K = np.zeros(tmax, dtype=np.int64)
    for j in range(tmax):
        K[j] = int((colcnt > j).sum(axis=2).max())
    for p in range(tmax // 2):
        K[2 * p + 1] = K[2 * p]
    K = [int(k) for k in K if k > 0]
    if len(K) % 2:
        K.append(K[-1])  # keep pairs aligned (all-pad plane)
    SH = int(sum(K))

    key = ("l2", tuple(K))
    if key not in _cache:
        _cache[key] = _build_l2(K)

    # cnt_corr per column = number of planes covering it (true edges + pads)
    Karr = np.asarray(K, dtype=np.int64)
    cols = np.arange(NHALF, dtype=np.int64)
    cnt_corr = (cols[None, :] < Karr[:, None]).sum(axis=0).astype(np.float32)

    A_np = np.asarray(A_T)          # bf16 [64, N]
    nB_np = np.asarray(negB_T)
    xbT = xb.T                      # bf16 [64, N]
    wu = np.zeros((128, 64), dtype=np.float32)
    wu[:64] = W_upd[:64]
    wu[64:] = W_upd[64:]
    wu = wu.astype(BF)
    bu = b_upd.reshape(64, 1).astype(np.float32)

    in2 = []
    for c in range(CORES):
        lo = c * NPC
        # per-half flat src-index stream
        G = np.empty((128, SH), dtype=BF)
        for h in range(2):
            nodes = colnode[c, h]
            ncnt = colcnt[c, h]
            starts = cum[nodes]
            pieces = []
            for j, kj in enumerate(K):
                nds = slice(0, kj)
                valid = ncnt[nds] > j
                idx = np.where(valid, starts[nds] + j, -1)
                pieces.append(idx)
            srcflat = np.concatenate(pieces)
            srcs = src[order[np.maximum(srcflat, 0)]]
            Gh = A_np[:, srcs]
            Gh[:, srcflat < 0] = PAD_G
            G[64 * h:64 * h + 64] = Gh
        nb2 = np.empty((128, NHALF), dtype=BF)
        ic2 = np.empty((128, NHALF), dtype=BF)
        rc2 = np.empty((128, NHALF), dtype=BF)
        xu2 = np.empty((128, NHALF), dtype=BF)
        for h in range(2):
            r = slice(64 * h, 64 * h + 64)
            nb2[r] = nB_np[:, colnode[c, h]]
            xu2[r] = xbT[:, colnode[c, h]]
            ic2[r] = (1.0 / np.maximum(colcnt[c, h], 1)).astype(BF)[None, :]
            rc2[r] = (-cnt_corr).astype(BF)[None, :]
        in2.append({"g": G, "nb": nb2, "ic": ic2, "rc": rc2, "xu": xu2,
                    "wu": wu, "bu": bu})

    res2 = run_bass_kernel_spmd(_cache[key], in2, list(range(CORES)))

    out = np.empty((N_NODES, 64), dtype=np.float32)
    for c in range(CORES):
        upd = np.asarray(res2.results[c]["upd"], dtype=np.float32)
        lo = c * NPC
        for h in range(2):
            loc = colloc[c, h]
            real = loc < NPC
            vals = upd[64 * h:64 * h + 64, :].T      # [NHALF, 64]
            out[lo + loc[real]] = vals[real]
    return out


# revision 62
# speedup vs baseline: 3.4594x; 3.4594x over previous
"""GNN message-passing layer on 8 trn2 NeuronCores.

Math: messages = relu(x_src@W1 + x_tgt@W2 + b); agg = mean over target;
out = relu(concat(x, agg) @ W_upd + bu).

Plan (target-sharded; host does index work, the A-row gather, and constant
prep only):
  L1 (device): per-core node shard -> A^T=(x@W1)^T and negB'^T=-(x@W2+b)^T
      in one K=66 matmul (ones-row folds the bias), bf16.
  Host: sorts edges by target, nodes by in-degree descending; builds the
      "plane" stream: plane j = the j-th edge slot of every node with
      degree > j, so every slice the device touches is packed.  Gathers A
      rows into that stream (pure data movement; ~2/3 bf16 for DVE 2x
      throughput, 1/3 fp8 to cut DMA bytes).
  L2 (device): Y = max(G, -B') on DVE (relu(G+B') = max(G,-B') + B'; the
      +B' is folded into the PSUM init cnt_corr*B', which also cancels pad
      slots exactly), then PE identity-matmul injection into a PSUM f32
      accumulator (1024-node-column chunks).  agg = acc * (1/count) (ACT
      copy + DVE mult), then the update MLP + relu runs per chunk,
      software-pipelined one chunk behind.

The plane schedule is baked into the NEFF at kernel() time from the actual
edge_index (one SPMD schedule = max across cores; pad slots absorb skew).
"""

import numpy as np
import ml_dtypes

import concourse.bacc as bacc
import concourse.mybir as mybir
import concourse.tile as tile
from concourse.bass_utils import run_bass_kernel_spmd
from concourse.masks import make_identity

N_NODES = 100000
N_EDGES = 1600000
CORES = 8
NPC = N_NODES // CORES          # 12500 nodes per core
NHALF = 6272                    # per-half columns (2*6272 = 12544 >= 12500)
NPAD1 = 12800                   # L1 padded cols (25 x 512)
CHUNKN = 1024                   # node-columns per PSUM accumulation chunk
SEG = 512                       # segment width
DTILE = 8192                    # stream DMA tile width (elements)
BF16_OF = (0, 1)                # slab indices mod BF16_MOD that go bf16
BF16_MOD = 3

bf16 = mybir.dt.bfloat16
f32 = mybir.dt.float32
fp8 = mybir.dt.float8e4
BF = ml_dtypes.bfloat16
F8 = ml_dtypes.float8_e4m3
# pad value: max(PAD, -B') must equal -B' for any B' (|B'| < 4), and stay
# finite in fp8e4m3 (max 240)
PAD_G = np.float32(-240.0)

_cache = {}


def _plane_schedule(K):
    """Shared host/device schedule with two streams (0=bf16, 1=fp8).

    Returns (chunks, segs, SH) with SH = [SH_bf16, SH_fp8].
    chunks: list of (a, b, slabs, parts) per node-column chunk.
      slabs: (dt, off) - one full-width pair slab ([s x (planeA ws|planeB ws)],
        ws = min(SEG, b-a)) at offset off of stream dt.
      parts: (s, ws, dt, off) - partial-pair segment pair at offset off
        (planeA at off, planeB at off+ws), covering columns [a+s, a+s+ws).
    segs: flat (plane_j, col0, ws, dt, off) for the host gather.
    Slabs never straddle DTILE boundaries; alignment gaps are pad slots no
    compute op reads."""
    npair = len(K) // 2
    chunks = []
    segs = []
    cur = [0, 0]
    sli = 0  # global slab ordinal for dtype assignment
    a = 0

    def align(dt, need):
        if cur[dt] // DTILE != (cur[dt] + need - 1) // DTILE:
            cur[dt] = ((cur[dt] // DTILE) + 1) * DTILE

    while a < NHALF:
        b = min(a + CHUNKN, NHALF)
        w_ch = b - a
        ws_f = min(SEG, w_ch)
        n_s = (w_ch + ws_f - 1) // ws_f
        slab = 2 * w_ch
        full = [p for p in range(npair) if K[2 * p] >= b]
        part = [p for p in range(npair) if a < K[2 * p] < b]
        parts = []
        for p in part:
            w = K[2 * p] - a
            s = 0
            while s < w:
                ws = min(SEG, w - s)
                align(1, 2 * ws)
                off = cur[1]
                parts.append((s, ws, 1, off))
                segs.append((2 * p, a + s, ws, 1, off))
                segs.append((2 * p + 1, a + s, ws, 1, off + ws))
                cur[1] += 2 * ws
                s += ws
        slabs = []
        for p in full:
            dt = 0 if (sli % BF16_MOD) in BF16_OF else 1
            sli += 1
            align(dt, slab)
            off = cur[dt]
            for si in range(n_s):
                o = off + si * 2 * ws_f
                segs.append((2 * p, a + si * ws_f, ws_f, dt, o))
                segs.append((2 * p + 1, a + si * ws_f, ws_f, dt, o + ws_f))
            slabs.append((dt, off))
            cur[dt] += slab
        chunks.append((a, b, slabs, parts))
        a = b
    SH = [((c + DTILE - 1) // DTILE) * DTILE for c in cur]
    return chunks, segs, SH


def _build_l1():
    nc = bacc.Bacc("TRN2", debug=False, num_devices=CORES)
    xt65 = nc.dram_tensor("xt65", [66, NPAD1], bf16, kind="ExternalInput")
    wab = nc.dram_tensor("wab", [66, 128], bf16, kind="ExternalInput")
    ab = nc.dram_tensor("ab", [128, NPAD1], bf16, kind="ExternalOutput")

    QW = 2560  # 5 tiles of 512 per DMA piece
    with tile.TileContext(nc) as tc:
        with (
            tc.tile_pool(name="big", bufs=1) as big,
            tc.tile_pool(name="psum", bufs=4, space="PSUM") as psum,
        ):
            wt = big.tile([66, 128], bf16)
            xt = big.tile([66, NPAD1], bf16)
            abt = big.tile([128, NPAD1], bf16)
            nc.sync.dma_start(out=wt[:], in_=wab[:, :])
            for q in range(NPAD1 // QW):
                qs = slice(q * QW, (q + 1) * QW)
                nc.sync.dma_start(out=xt[:, qs], in_=xt65[:, qs])
            for c in range(NPAD1 // 512):
                sl = slice(c * 512, (c + 1) * 512)
                pt = psum.tile([128, 512], f32)
                nc.tensor.matmul(out=pt[:], lhsT=wt[:], rhs=xt[:, sl],
                                 start=True, stop=True)
                if c % 2 == 0:
                    nc.vector.tensor_copy(out=abt[:, sl], in_=pt[:])
                else:
                    nc.scalar.activation(
                        out=abt[:, sl], in_=pt[:],
                        func=mybir.ActivationFunctionType.Copy)
                if (c + 1) % 5 == 0:
                    qs = slice((c + 1) * 512 - QW, (c + 1) * 512)
                    nc.sync.dma_start(out=ab[:, qs], in_=abt[:, qs])
    nc.compile()
    return nc


def _build_l2(K):
    chunks, _segs, SH = _plane_schedule(K)
    nc = bacc.Bacc("TRN2", debug=False, num_devices=CORES)
    g16 = nc.dram_tensor("g16", [128, SH[0]], bf16, kind="ExternalInput")
    g8 = nc.dram_tensor("g8", [128, SH[1]], fp8, kind="ExternalInput")
    nb = nc.dram_tensor("nb", [128, NHALF], bf16, kind="ExternalInput")
    ic = nc.dram_tensor("ic", [128, NHALF], bf16, kind="ExternalInput")
    ini = nc.dram_tensor("ini", [128, NHALF], bf16, kind="ExternalInput")
    xu = nc.dram_tensor("xu", [128, NHALF], bf16, kind="ExternalInput")
    wu = nc.dram_tensor("wu", [128, 128], bf16, kind="ExternalInput")
    bu = nc.dram_tensor("bu", [64, 1], f32, kind="ExternalInput")
    upd = nc.dram_tensor("upd", [128, NHALF], bf16, kind="ExternalOutput")

    amax = mybir.AluOpType.max
    mult = mybir.AluOpType.mult
    gdram = (g16, g8)
    gdt = (bf16, fp8)
    ntile = (SH[0] // DTILE, SH[1] // DTILE)

    with tile.TileContext(nc) as tc:
        with (
            tc.tile_pool(name="persist", bufs=1) as per,
            tc.tile_pool(name="st16", bufs=3) as st16p,
            tc.tile_pool(name="st8", bufs=3) as st8p,
            tc.tile_pool(name="ybuf", bufs=4) as ybuf,
            tc.tile_pool(name="ypart", bufs=6) as ypart,
            tc.tile_pool(name="abuf", bufs=3) as abuf,
            tc.tile_pool(name="obuf", bufs=3) as obuf,
            tc.tile_pool(name="acc", bufs=2, space="PSUM") as accp,
            tc.tile_pool(name="upsum", bufs=2, space="PSUM") as upsum,
        ):
            nb_t = per.tile([128, NHALF], bf16)
            ic_t = per.tile([128, NHALF], bf16)
            ini_t = per.tile([128, NHALF], bf16)
            xu_t = per.tile([128, NHALF], bf16)
            wu_t = per.tile([128, 128], bf16)
            bu_t = per.tile([64, 1], f32)
            agg_t = per.tile([128, NHALF], bf16)
            ident = per.tile([128, 128], bf16)
            nc.scalar.dma_start(out=wu_t[:], in_=wu[:, :])
            nc.scalar.dma_start(out=bu_t[:], in_=bu[:, :])
            make_identity(nc, ident[:])

            st_tiles = [{}, {}]
            stp = (st16p, st8p)

            def stile(dt, i):
                cachebin = st_tiles[dt]
                if i not in cachebin:
                    t = stp[dt].tile([128, DTILE], gdt[dt], tag="st")
                    h = DTILE // 2
                    nc.sync.dma_start(
                        out=t[:, 0:h],
                        in_=gdram[dt][:, i * DTILE:i * DTILE + h])
                    nc.sync.dma_start(
                        out=t[:, h:],
                        in_=gdram[dt][:, i * DTILE + h:(i + 1) * DTILE])
                    cachebin[i] = t
                return cachebin[i]

            def prologue(ci):
                a, b, _s, _p = chunks[ci]
                nc.sync.dma_start(out=nb_t[:, a:b], in_=nb[:, a:b])
                nc.sync.dma_start(out=ini_t[:, a:b], in_=ini[:, a:b])

            def finish(a, b, acc_t):
                # per-tile: SBUF copy of acc (ACT), agg = copy * (1/count)
                # (DVE), update MLP (PE), relu+bias (ACT), out DMA
                t0 = a
                qi = 0
                while t0 < b:
                    w = min(SEG, b - t0)
                    sl = slice(t0, t0 + w)
                    at = abuf.tile([128, SEG], bf16, tag="at")
                    nc.scalar.activation(
                        out=at[:, 0:w], in_=acc_t[:, t0 - a:t0 - a + w],
                        func=mybir.ActivationFunctionType.Copy)
                    nc.vector.tensor_tensor(out=agg_t[:, sl], in0=at[:, 0:w],
                                            in1=ic_t[:, sl], op=mult)
                    for h in range(2):
                        ps = slice(64 * h, 64 * h + 64)
                        pt = upsum.tile([64, SEG], f32, tag=f"pt{h}")
                        nc.tensor.matmul(out=pt[:, 0:w], lhsT=wu_t[ps, 0:64],
                                         rhs=xu_t[ps, sl],
                                         start=True, stop=False)
                        nc.tensor.matmul(out=pt[:, 0:w],
                                         lhsT=wu_t[ps, 64:128],
                                         rhs=agg_t[ps, sl],
                                         start=False, stop=True)
                        ot = obuf.tile([64, SEG], bf16, tag=f"ot{h}")
                        nc.scalar.activation(
                            out=ot[:, 0:w], in_=pt[:, 0:w],
                            func=mybir.ActivationFunctionType.Relu,
                            bias=bu_t[:])
                        q = (nc.sync, nc.scalar)[qi % 2]
                        qi += 1
                        q.dma_start(out=upd[ps, sl], in_=ot[:, 0:w])
                    t0 += w

            prologue(0)
            prev = None
            for ci, (a, b, slabs, parts) in enumerate(chunks):
                w_ch = b - a
                ws_f = min(SEG, w_ch)
                n_s = (w_ch + ws_f - 1) // ws_f
                slab = 2 * w_ch
                acc_t = accp.tile([128, CHUNKN], f32, tag="acc")
                # ISA: matmul rhs <= [128, 512]
                for s0 in range(0, w_ch, SEG):
                    w = min(SEG, w_ch - s0)
                    nc.tensor.matmul(out=acc_t[:, s0:s0 + w], lhsT=ident[:],
                                     rhs=ini_t[:, a + s0:a + s0 + w],
                                     start=True, stop=False)
                if ci + 1 < len(chunks):
                    prologue(ci + 1)
                n_inj = 2 * n_s * len(slabs) + 2 * len(parts)
                inj = 0

                def inject(rhs_ap, s0, ws, last):
                    nc.tensor.matmul(out=acc_t[:, s0:s0 + ws], lhsT=ident[:],
                                     rhs=rhs_ap, start=False, stop=last)

                for (s, ws, dt, off) in parts:
                    st = stile(dt, off // DTILE)
                    la = off % DTILE
                    cols = slice(a + s, a + s + ws)
                    yt = ypart.tile([128, 2 * SEG], bf16, tag="yp")
                    nbb = nb_t[:, cols].unsqueeze(1).to_broadcast([128, 2, ws])
                    nc.vector.tensor_tensor(out=yt[:, 0:2 * ws],
                                            in0=st[:, la:la + 2 * ws],
                                            in1=nbb, op=amax)
                    inj += 2
                    inject(yt[:, 0:ws], s, ws, False)
                    inject(yt[:, ws:2 * ws], s, ws, inj == n_inj)
                # -B' pattern matching one slab's [s x (A|B)] layout
                base = nb_t[:, a:b]
                if n_s > 1:
                    v = base.rearrange("p (s c) -> p s c", s=n_s)
                    v = v.unsqueeze(2).to_broadcast([128, n_s, 2, ws_f])
                else:
                    v = base.unsqueeze(1).to_broadcast([128, 2, w_ch])
                for si_, (dt, off) in enumerate(slabs):
                    ti = off // DTILE
                    la = off % DTILE
                    st = stile(dt, ti)
                    if ti + 1 < ntile[dt]:
                        stile(dt, ti + 1)  # prefetch
                    if si_ == 2 and prev is not None:
                        # software pipeline: previous chunk's finish behind
                        # this chunk's first slabs
                        finish(*prev)
                        prev = None
                    yt = ybuf.tile([128, 2 * CHUNKN], bf16, tag="yt")
                    nc.vector.tensor_tensor(out=yt[:, 0:slab],
                                            in0=st[:, la:la + slab],
                                            in1=v, op=amax)
                    for si in range(n_s):
                        lo = si * 2 * ws_f
                        inj += 2
                        inject(yt[:, lo:lo + ws_f], si * ws_f, ws_f, False)
                        inject(yt[:, lo + ws_f:lo + 2 * ws_f],
                               si * ws_f, ws_f, inj == n_inj)
                if prev is not None:
                    finish(*prev)
                # update-phase inputs, consumed by finish() one chunk later
                nc.scalar.dma_start(out=ic_t[:, a:b], in_=ic[:, a:b])
                nc.scalar.dma_start(out=xu_t[:, a:b], in_=xu[:, a:b])
                prev = (a, b, acc_t)
            finish(*prev)
    nc.compile()
    return nc


def kernel(x, edge_index, W_msg, b_msg, W_upd, b_upd):
    x = np.asarray(x, dtype=np.float32)
    src = np.asarray(edge_index[0], dtype=np.int64)
    tgt = np.asarray(edge_index[1], dtype=np.int64)
    W_msg = np.asarray(W_msg, dtype=np.float32)
    b_msg = np.asarray(b_msg, dtype=np.float32)
    W_upd = np.asarray(W_upd, dtype=np.float32)
    b_upd = np.asarray(b_upd, dtype=np.float32)

    # ---------------- L1 ----------------
    if "l1" not in _cache:
        _cache["l1"] = _build_l1()
    wab = np.zeros((66, 128), dtype=np.float32)
    wab[:64, :64] = W_msg[:64]
    wab[:64, 64:] = -W_msg[64:]
    wab[64, 64:] = -b_msg
    wab = wab.astype(BF)
    xb = x.astype(BF)
    in1 = []
    for c in range(CORES):
        xt65 = np.zeros((66, NPAD1), dtype=BF)
        xt65[:64, :NPC] = xb[c * NPC:(c + 1) * NPC].T
        xt65[64, :] = np.float32(1.0)
        in1.append({"xt65": xt65, "wab": wab})
    res1 = run_bass_kernel_spmd(_cache["l1"], in1, list(range(CORES)))
    A_T = np.concatenate([np.asarray(r["ab"])[0:64, :NPC]
                          for r in res1.results], axis=1)
    negB_T = np.concatenate([np.asarray(r["ab"])[64:128, :NPC]
                             for r in res1.results], axis=1)

    # ---------------- host: plane schedule ----------------
    counts = np.bincount(tgt, minlength=N_NODES).astype(np.int64)
    order = np.argsort(tgt, kind="stable")
    cum = np.zeros(N_NODES + 1, dtype=np.int64)
    np.cumsum(counts, out=cum[1:])

    colloc = np.empty((CORES, 2, NHALF), dtype=np.int64)
    colnode = np.empty((CORES, 2, NHALF), dtype=np.int64)
    colcnt = np.zeros((CORES, 2, NHALF), dtype=np.int64)
    for c in range(CORES):
        lo = c * NPC
        cnt_loc = np.zeros(2 * NHALF, dtype=np.int64)
        cnt_loc[:NPC] = counts[lo:lo + NPC]
        rank = np.argsort(-cnt_loc, kind="stable")
        for h in range(2):
            nodes = rank[h::2]
            colloc[c, h] = nodes
            colnode[c, h] = np.minimum(nodes, NPC - 1) + lo
            colcnt[c, h] = cnt_loc[nodes]

    tmax = int(colcnt.max())
    tmax += tmax % 2
    K = np.zeros(tmax, dtype=np.int64)
    for j in range(tmax):
        K[j] = int((colcnt > j).sum(axis=2).max())
    for p in range(tmax // 2):
        K[2 * p + 1] = K[2 * p]
    K = [int(k) for k in K if k > 0]
    if len(K) % 2:
        K.append(K[-1])

    key = ("l2", tuple(K))
    if key not in _cache:
        _cache[key] = _build_l2(K)
    chunks, segs, SH = _plane_schedule(K)

    # cnt_corr: true edges + pad slots per column = planes covering the col
    Karr = np.asarray(K, dtype=np.int64)
    cols = np.arange(NHALF, dtype=np.int64)
    cnt_corr = (cols[None, :] < Karr[:, None]).sum(axis=0).astype(np.float32)

    A_np = np.asarray(A_T)
    A8 = A_np.astype(F8)
    nB_np = np.asarray(negB_T)
    xbT = np.ascontiguousarray(xb.T)
    wu = np.zeros((128, 128), dtype=np.float32)
    wu[:64, :64] = W_upd[:64]
    wu[:64, 64:] = W_upd[64:]
    wu[64:] = wu[:64]
    wu = wu.astype(BF)
    bu = b_upd.reshape(64, 1).astype(np.float32)

    in2 = []
    for c in range(CORES):
        G16 = np.full((128, SH[0]), PAD_G, dtype=BF)
        G8 = np.full((128, SH[1]), PAD_G, dtype=F8)
        for h in range(2):
            nodes = colnode[c, h]
            ncnt = colcnt[c, h]
            starts = cum[nodes]
            for dt, Gt, At in ((0, G16, A_np), (1, G8, A8)):
                srcflat = np.full(SH[dt], -1, dtype=np.int64)
                for (jj, col0, ws, sdt, off) in segs:
                    if sdt != dt:
                        continue
                    csl = slice(col0, col0 + ws)
                    valid = ncnt[csl] > jj
                    srcflat[off:off + ws] = np.where(
                        valid, starts[csl] + jj, -1)
                have = srcflat >= 0
                idx = src[order[srcflat[have]]]
                Gh = np.full((64, SH[dt]), PAD_G, dtype=Gt.dtype)
                Gh[:, have] = At[:, idx]
                Gt[64 * h:64 * h + 64] = Gh
        nb2 = np.empty((128, NHALF), dtype=BF)
        ic2 = np.empty((128, NHALF), dtype=BF)
        ini2 = np.empty((128, NHALF), dtype=BF)
        xu2 = np.empty((128, NHALF), dtype=BF)
        for h in range(2):
            r = slice(64 * h, 64 * h + 64)
            nbh = nB_np[:, colnode[c, h]]
            nb2[r] = nbh
            xu2[r] = xbT[:, colnode[c, h]]
            ic2[r] = (1.0 / np.maximum(colcnt[c, h], 1)).astype(BF)[None, :]
            # ini = cnt_corr * B' = (-cnt_corr) * (-B')
            ini2[r] = ((-cnt_corr[None, :]) *
                       nbh.astype(np.float32)).astype(BF)
        in2.append({"g16": G16, "g8": G8, "nb": nb2, "ic": ic2, "ini": ini2,
                    "xu": xu2, "wu": wu, "bu": bu})

    res2 = run_bass_kernel_spmd(_cache[key], in2, list(range(CORES)))

    out = np.empty((N_NODES, 64), dtype=np.float32)
    for c in range(CORES):
        upd = np.asarray(res2.results[c]["upd"]).astype(np.float32)
        lo = c * NPC
        for h in range(2):
            loc = colloc[c, h]
            real = loc < NPC
            vals = upd[64 * h:64 * h + 64, :].T
            out[lo + loc[real]] = vals[real]
    return out


# revision 64
# speedup vs baseline: 3.5573x; 1.0283x over previous
"""GNN message-passing layer on 8 trn2 NeuronCores.

Math: messages = relu(x_src@W1 + x_tgt@W2 + b); agg = mean over target;
out = relu(concat(x, agg) @ W_upd + bu).

Plan (target-sharded; host does index work, the A-row gather, and constant
prep only):
  L1 (device): per-core node shard -> A^T=(x@W1)^T and negB'^T=-(x@W2+b)^T
      in one K=66 matmul (ones-row folds the bias), bf16.
  Host: sorts edges by target, nodes by in-degree descending; builds the
      "plane" stream: plane j = the j-th edge slot of every node with
      degree > j, so every slice the device touches is packed.  Gathers A
      rows into that stream (pure data movement; ~2/3 bf16 for DVE 2x
      throughput, 1/3 fp8 to cut DMA bytes).
  L2 (device): Y = max(G, -B') on DVE (relu(G+B') = max(G,-B') + B'; the
      +B' is folded into the PSUM init cnt_corr*B', which also cancels pad
      slots exactly), then PE identity-matmul injection into a PSUM f32
      accumulator (1024-node-column chunks).  agg = acc * (1/count) (ACT
      copy + DVE mult), then the update MLP + relu runs per chunk,
      software-pipelined one chunk behind.

The plane schedule is baked into the NEFF at kernel() time from the actual
edge_index (one SPMD schedule = max across cores; pad slots absorb skew).
"""

import numpy as np
import ml_dtypes

import concourse.bacc as bacc
import concourse.mybir as mybir
import concourse.tile as tile
from concourse.bass_utils import run_bass_kernel_spmd
from concourse.masks import make_identity

N_NODES = 100000
N_EDGES = 1600000
CORES = 8
NPC = N_NODES // CORES          # 12500 nodes per core
NHALF = 6272                    # per-half columns (2*6272 = 12544 >= 12500)
NPAD1 = 12800                   # L1 padded cols (25 x 512)
CHUNKN = 1024                   # node-columns per PSUM accumulation chunk
SEG = 512                       # segment width
DTILE = 8192                    # stream DMA tile width (elements)
# slab assignment pattern: (dtype, path) per slab ordinal mod len(pattern).
# dtype: 0=bf16, 1=fp8.  path: 0 = max(G,-B') on DVE; 1 = relu via PE-inject
# + ACT (PSUM staging), which frees DVE at the cost of PE/ACT work.
SLAB_PATTERN = ((0, 0),) * 12 + ((1, 0),) * 3 + ((1, 1),) * 5

bf16 = mybir.dt.bfloat16
f32 = mybir.dt.float32
fp8 = mybir.dt.float8e4
BF = ml_dtypes.bfloat16
F8 = ml_dtypes.float8_e4m3
# pad value: max(PAD, -B') must equal -B' for any B' (|B'| < 4), and stay
# finite in fp8e4m3 (max 240)
PAD_G = np.float32(-240.0)

_cache = {}


def _plane_schedule(K):
    """Shared host/device schedule with two streams (0=bf16, 1=fp8).

    Returns (chunks, segs, SH) with SH = [SH_bf16, SH_fp8].
    chunks: list of (a, b, slabs, parts) per node-column chunk.
      slabs: (dt, off) - one full-width pair slab ([s x (planeA ws|planeB ws)],
        ws = min(SEG, b-a)) at offset off of stream dt.
      parts: (s, ws, dt, off) - partial-pair segment pair at offset off
        (planeA at off, planeB at off+ws), covering columns [a+s, a+s+ws).
    segs: flat (plane_j, col0, ws, dt, off) for the host gather.
    Slabs never straddle DTILE boundaries; alignment gaps are pad slots no
    compute op reads."""
    npair = len(K) // 2
    chunks = []
    segs = []
    cur = [0, 0]
    sli = 0  # global slab ordinal for dtype assignment
    a = 0

    def align(dt, need):
        if cur[dt] // DTILE != (cur[dt] + need - 1) // DTILE:
            cur[dt] = ((cur[dt] // DTILE) + 1) * DTILE

    while a < NHALF:
        b = min(a + CHUNKN, NHALF)
        w_ch = b - a
        ws_f = min(SEG, w_ch)
        n_s = (w_ch + ws_f - 1) // ws_f
        slab = 2 * w_ch
        full = [p for p in range(npair) if K[2 * p] >= b]
        part = [p for p in range(npair) if a < K[2 * p] < b]
        parts = []
        for p in part:
            w = K[2 * p] - a
            s = 0
            while s < w:
                ws = min(SEG, w - s)
                align(1, 2 * ws)
                off = cur[1]
                parts.append((s, ws, 1, off))
                segs.append((2 * p, a + s, ws, 1, off, 0))
                segs.append((2 * p + 1, a + s, ws, 1, off + ws, 0))
                cur[1] += 2 * ws
                s += ws
        slabs = []
        for p in full:
            dt, path = SLAB_PATTERN[sli % len(SLAB_PATTERN)]
            sli += 1
            align(dt, slab)
            off = cur[dt]
            for si in range(n_s):
                o = off + si * 2 * ws_f
                segs.append((2 * p, a + si * ws_f, ws_f, dt, o, path))
                segs.append((2 * p + 1, a + si * ws_f, ws_f, dt, o + ws_f,
                             path))
            slabs.append((dt, off, path))
            cur[dt] += slab
        chunks.append((a, b, slabs, parts))
        a = b
    SH = [((c + DTILE - 1) // DTILE) * DTILE for c in cur]
    return chunks, segs, SH


def _build_l1():
    nc = bacc.Bacc("TRN2", debug=False, num_devices=CORES)
    xt65 = nc.dram_tensor("xt65", [66, NPAD1], bf16, kind="ExternalInput")
    wab = nc.dram_tensor("wab", [66, 128], bf16, kind="ExternalInput")
    ab = nc.dram_tensor("ab", [128, NPAD1], bf16, kind="ExternalOutput")

    QW = 2560  # 5 tiles of 512 per DMA piece
    with tile.TileContext(nc) as tc:
        with (
            tc.tile_pool(name="big", bufs=1) as big,
            tc.tile_pool(name="psum", bufs=4, space="PSUM") as psum,
        ):
            wt = big.tile([66, 128], bf16)
            xt = big.tile([66, NPAD1], bf16)
            abt = big.tile([128, NPAD1], bf16)
            nc.sync.dma_start(out=wt[:], in_=wab[:, :])
            for q in range(NPAD1 // QW):
                qs = slice(q * QW, (q + 1) * QW)
                nc.sync.dma_start(out=xt[:, qs], in_=xt65[:, qs])
            for c in range(NPAD1 // 512):
                sl = slice(c * 512, (c + 1) * 512)
                pt = psum.tile([128, 512], f32)
                nc.tensor.matmul(out=pt[:], lhsT=wt[:], rhs=xt[:, sl],
                                 start=True, stop=True)
                if c % 2 == 0:
                    nc.vector.tensor_copy(out=abt[:, sl], in_=pt[:])
                else:
                    nc.scalar.activation(
                        out=abt[:, sl], in_=pt[:],
                        func=mybir.ActivationFunctionType.Copy)
                if (c + 1) % 5 == 0:
                    qs = slice((c + 1) * 512 - QW, (c + 1) * 512)
                    nc.sync.dma_start(out=ab[:, qs], in_=abt[:, qs])
    nc.compile()
    return nc


def _build_l2(K):
    chunks, _segs, SH = _plane_schedule(K)
    nc = bacc.Bacc("TRN2", debug=False, num_devices=CORES)
    g16 = nc.dram_tensor("g16", [128, SH[0]], bf16, kind="ExternalInput")
    g8 = nc.dram_tensor("g8", [128, SH[1]], fp8, kind="ExternalInput")
    nb = nc.dram_tensor("nb", [128, NHALF], bf16, kind="ExternalInput")
    ic = nc.dram_tensor("ic", [128, NHALF], bf16, kind="ExternalInput")
    ini = nc.dram_tensor("ini", [128, NHALF], bf16, kind="ExternalInput")
    xu = nc.dram_tensor("xu", [128, NHALF], bf16, kind="ExternalInput")
    wu = nc.dram_tensor("wu", [128, 128], bf16, kind="ExternalInput")
    bu = nc.dram_tensor("bu", [64, 1], f32, kind="ExternalInput")
    upd = nc.dram_tensor("upd", [128, NHALF], bf16, kind="ExternalOutput")

    amax = mybir.AluOpType.max
    mult = mybir.AluOpType.mult
    gdram = (g16, g8)
    gdt = (bf16, fp8)
    ntile = (SH[0] // DTILE, SH[1] // DTILE)

    with tile.TileContext(nc) as tc:
        with (
            tc.tile_pool(name="persist", bufs=1) as per,
            tc.tile_pool(name="st16", bufs=3) as st16p,
            tc.tile_pool(name="st8", bufs=3) as st8p,
            tc.tile_pool(name="ybuf", bufs=4) as ybuf,
            tc.tile_pool(name="ypart", bufs=6) as ypart,
            tc.tile_pool(name="abuf", bufs=3) as abuf,
            tc.tile_pool(name="obuf", bufs=3) as obuf,
            tc.tile_pool(name="acc", bufs=2, space="PSUM") as accp,
            tc.tile_pool(name="upsum", bufs=2, space="PSUM") as upsum,
        ):
            nb_t = per.tile([128, NHALF], bf16)
            ic_t = per.tile([128, NHALF], bf16)
            ini_t = per.tile([128, NHALF], bf16)
            xu_t = per.tile([128, NHALF], bf16)
            wu_t = per.tile([128, 128], bf16)
            bu_t = per.tile([64, 1], f32)
            agg_t = per.tile([128, NHALF], bf16)
            ident = per.tile([128, 128], bf16)
            nc.scalar.dma_start(out=wu_t[:], in_=wu[:, :])
            nc.scalar.dma_start(out=bu_t[:], in_=bu[:, :])
            make_identity(nc, ident[:])

            st_tiles = [{}, {}]
            stp = (st16p, st8p)

            def stile(dt, i):
                cachebin = st_tiles[dt]
                if i not in cachebin:
                    t = stp[dt].tile([128, DTILE], gdt[dt], tag="st")
                    h = DTILE // 2
                    nc.sync.dma_start(
                        out=t[:, 0:h],
                        in_=gdram[dt][:, i * DTILE:i * DTILE + h])
                    nc.sync.dma_start(
                        out=t[:, h:],
                        in_=gdram[dt][:, i * DTILE + h:(i + 1) * DTILE])
                    cachebin[i] = t
                return cachebin[i]

            def prologue(ci):
                a, b, _s, _p = chunks[ci]
                nc.sync.dma_start(out=nb_t[:, a:b], in_=nb[:, a:b])
                nc.sync.dma_start(out=ini_t[:, a:b], in_=ini[:, a:b])

            def finish(a, b, acc_t):
                # per-tile: SBUF copy of acc (ACT), agg = copy * (1/count)
                # (DVE), update MLP (PE), relu+bias (ACT), out DMA
                t0 = a
                qi = 0
                while t0 < b:
                    w = min(SEG, b - t0)
                    sl = slice(t0, t0 + w)
                    at = abuf.tile([128, SEG], bf16, tag="at")
                    nc.scalar.activation(
                        out=at[:, 0:w], in_=acc_t[:, t0 - a:t0 - a + w],
                        func=mybir.ActivationFunctionType.Copy)
                    nc.vector.tensor_tensor(out=agg_t[:, sl], in0=at[:, 0:w],
                                            in1=ic_t[:, sl], op=mult)
                    for h in range(2):
                        ps = slice(64 * h, 64 * h + 64)
                        pt = upsum.tile([64, SEG], f32, tag=f"pt{h}")
                        nc.tensor.matmul(out=pt[:, 0:w], lhsT=wu_t[ps, 0:64],
                                         rhs=xu_t[ps, sl],
                                         start=True, stop=False)
                        nc.tensor.matmul(out=pt[:, 0:w],
                                         lhsT=wu_t[ps, 64:128],
                                         rhs=agg_t[ps, sl],
                                         start=False, stop=True)
                        ot = obuf.tile([64, SEG], bf16, tag=f"ot{h}")
                        nc.scalar.activation(
                            out=ot[:, 0:w], in_=pt[:, 0:w],
                            func=mybir.ActivationFunctionType.Relu,
                            bias=bu_t[:])
                        q = (nc.sync, nc.scalar)[qi % 2]
                        qi += 1
                        q.dma_start(out=upd[ps, sl], in_=ot[:, 0:w])
                    t0 += w

            prologue(0)
            prev = None
            for ci, (a, b, slabs, parts) in enumerate(chunks):
                w_ch = b - a
                ws_f = min(SEG, w_ch)
                n_s = (w_ch + ws_f - 1) // ws_f
                slab = 2 * w_ch
                acc_t = accp.tile([128, CHUNKN], f32, tag="acc")
                # ISA: matmul rhs <= [128, 512]
                for s0 in range(0, w_ch, SEG):
                    w = min(SEG, w_ch - s0)
                    nc.tensor.matmul(out=acc_t[:, s0:s0 + w], lhsT=ident[:],
                                     rhs=ini_t[:, a + s0:a + s0 + w],
                                     start=True, stop=False)
                if ci + 1 < len(chunks):
                    prologue(ci + 1)
                n_inj = 2 * n_s * len(slabs) + 2 * len(parts)
                inj = 0

                def inject(rhs_ap, s0, ws, last):
                    nc.tensor.matmul(out=acc_t[:, s0:s0 + ws], lhsT=ident[:],
                                     rhs=rhs_ap, start=False, stop=last)

                for (s, ws, dt, off) in parts:
                    st = stile(dt, off // DTILE)
                    la = off % DTILE
                    cols = slice(a + s, a + s + ws)
                    yt = ypart.tile([128, 2 * SEG], bf16, tag="yp")
                    nbb = nb_t[:, cols].unsqueeze(1).to_broadcast([128, 2, ws])
                    nc.vector.tensor_tensor(out=yt[:, 0:2 * ws],
                                            in0=st[:, la:la + 2 * ws],
                                            in1=nbb, op=amax)
                    inj += 2
                    inject(yt[:, 0:ws], s, ws, False)
                    inject(yt[:, ws:2 * ws], s, ws, inj == n_inj)
                # -B' pattern matching one slab's [s x (A|B)] layout
                base = nb_t[:, a:b]
                if n_s > 1:
                    v = base.rearrange("p (s c) -> p s c", s=n_s)
                    v = v.unsqueeze(2).to_broadcast([128, n_s, 2, ws_f])
                else:
                    v = base.unsqueeze(1).to_broadcast([128, 2, w_ch])
                for si_, (dt, off) in enumerate(slabs):
                    ti = off // DTILE
                    la = off % DTILE
                    st = stile(dt, ti)
                    if ti + 1 < ntile[dt]:
                        stile(dt, ti + 1)  # prefetch
                    if si_ == 2 and prev is not None:
                        # software pipeline: previous chunk's finish behind
                        # this chunk's first slabs
                        finish(*prev)
                        prev = None
                    yt = ybuf.tile([128, 2 * CHUNKN], bf16, tag="yt")
                    nc.vector.tensor_tensor(out=yt[:, 0:slab],
                                            in0=st[:, la:la + slab],
                                            in1=v, op=amax)
                    for si in range(n_s):
                        lo = si * 2 * ws_f
                        inj += 2
                        inject(yt[:, lo:lo + ws_f], si * ws_f, ws_f, False)
                        inject(yt[:, lo + ws_f:lo + 2 * ws_f],
                               si * ws_f, ws_f, inj == n_inj)
                if prev is not None:
                    finish(*prev)
                # update-phase inputs, consumed by finish() one chunk later
                nc.scalar.dma_start(out=ic_t[:, a:b], in_=ic[:, a:b])
                nc.scalar.dma_start(out=xu_t[:, a:b], in_=xu[:, a:b])
                prev = (a, b, acc_t)
            finish(*prev)
    nc.compile()
    return nc


def kernel(x, edge_index, W_msg, b_msg, W_upd, b_upd):
    x = np.asarray(x, dtype=np.float32)
    src = np.asarray(edge_index[0], dtype=np.int64)
    tgt = np.asarray(edge_index[1], dtype=np.int64)
    W_msg = np.asarray(W_msg, dtype=np.float32)
    b_msg = np.asarray(b_msg, dtype=np.float32)
    W_upd = np.asarray(W_upd, dtype=np.float32)
    b_upd = np.asarray(b_upd, dtype=np.float32)

    # ---------------- L1 ----------------
    if "l1" not in _cache:
        _cache["l1"] = _build_l1()
    wab = np.zeros((66, 128), dtype=np.float32)
    wab[:64, :64] = W_msg[:64]
    wab[:64, 64:] = -W_msg[64:]
    wab[64, 64:] = -b_msg
    wab = wab.astype(BF)
    xb = x.astype(BF)
    in1 = []
    for c in range(CORES):
        xt65 = np.zeros((66, NPAD1), dtype=BF)
        xt65[:64, :NPC] = xb[c * NPC:(c + 1) * NPC].T
        xt65[64, :] = np.float32(1.0)
        in1.append({"xt65": xt65, "wab": wab})
    res1 = run_bass_kernel_spmd(_cache["l1"], in1, list(range(CORES)))
    A_T = np.concatenate([np.asarray(r["ab"])[0:64, :NPC]
                          for r in res1.results], axis=1)
    negB_T = np.concatenate([np.asarray(r["ab"])[64:128, :NPC]
                             for r in res1.results], axis=1)

    # ---------------- host: plane schedule ----------------
    counts = np.bincount(tgt, minlength=N_NODES).astype(np.int64)
    order = np.argsort(tgt, kind="stable")
    cum = np.zeros(N_NODES + 1, dtype=np.int64)
    np.cumsum(counts, out=cum[1:])

    colloc = np.empty((CORES, 2, NHALF), dtype=np.int64)
    colnode = np.empty((CORES, 2, NHALF), dtype=np.int64)
    colcnt = np.zeros((CORES, 2, NHALF), dtype=np.int64)
    for c in range(CORES):
        lo = c * NPC
        cnt_loc = np.zeros(2 * NHALF, dtype=np.int64)
        cnt_loc[:NPC] = counts[lo:lo + NPC]
        rank = np.argsort(-cnt_loc, kind="stable")
        for h in range(2):
            nodes = rank[h::2]
            colloc[c, h] = nodes
            colnode[c, h] = np.minimum(nodes, NPC - 1) + lo
            colcnt[c, h] = cnt_loc[nodes]

    tmax = int(colcnt.max())
    tmax += tmax % 2
    K = np.zeros(tmax, dtype=np.int64)
    for j in range(tmax):
        K[j] = int((colcnt > j).sum(axis=2).max())
    for p in range(tmax // 2):
        K[2 * p + 1] = K[2 * p]
    K = [int(k) for k in K if k > 0]
    if len(K) % 2:
        K.append(K[-1])

    key = ("l2", tuple(K))
    if key not in _cache:
        _cache[key] = _build_l2(K)
    chunks, segs, SH = _plane_schedule(K)

    # cnt_corr: true edges + pad slots per column = planes covering the col
    Karr = np.asarray(K, dtype=np.int64)
    cols = np.arange(NHALF, dtype=np.int64)
    cnt_corr = (cols[None, :] < Karr[:, None]).sum(axis=0).astype(np.float32)

    A_np = np.asarray(A_T)
    A8 = A_np.astype(F8)
    nB_np = np.asarray(negB_T)
    xbT = np.ascontiguousarray(xb.T)
    wu = np.zeros((128, 128), dtype=np.float32)
    wu[:64, :64] = W_upd[:64]
    wu[:64, 64:] = W_upd[64:]
    wu[64:] = wu[:64]
    wu = wu.astype(BF)
    bu = b_upd.reshape(64, 1).astype(np.float32)

    in2 = []
    for c in range(CORES):
        G16 = np.full((128, SH[0]), PAD_G, dtype=BF)
        G8 = np.full((128, SH[1]), PAD_G, dtype=F8)
        for h in range(2):
            nodes = colnode[c, h]
            ncnt = colcnt[c, h]
            starts = cum[nodes]
            for dt, Gt, At in ((0, G16, A_np), (1, G8, A8)):
                srcflat = np.full(SH[dt], -1, dtype=np.int64)
                for (jj, col0, ws, sdt, off) in segs:
                    if sdt != dt:
                        continue
                    csl = slice(col0, col0 + ws)
                    valid = ncnt[csl] > jj
                    srcflat[off:off + ws] = np.where(
                        valid, starts[csl] + jj, -1)
                have = srcflat >= 0
                idx = src[order[srcflat[have]]]
                Gh = np.full((64, SH[dt]), PAD_G, dtype=Gt.dtype)
                Gh[:, have] = At[:, idx]
                Gt[64 * h:64 * h + 64] = Gh
        nb2 = np.empty((128, NHALF), dtype=BF)
        ic2 = np.empty((128, NHALF), dtype=BF)
        ini2 = np.empty((128, NHALF), dtype=BF)
        xu2 = np.empty((128, NHALF), dtype=BF)
        for h in range(2):
            r = slice(64 * h, 64 * h + 64)
            nbh = nB_np[:, colnode[c, h]]
            nb2[r] = nbh
            xu2[r] = xbT[:, colnode[c, h]]
            ic2[r] = (1.0 / np.maximum(colcnt[c, h], 1)).astype(BF)[None, :]
            # ini = cnt_corr * B' = (-cnt_corr) * (-B')
            ini2[r] = ((-cnt_corr[None, :]) *
                       nbh.astype(np.float32)).astype(BF)
        in2.append({"g16": G16, "g8": G8, "nb": nb2, "ic": ic2, "ini": ini2,
                    "xu": xu2, "wu": wu, "bu": bu})

    res2 = run_bass_kernel_spmd(_cache[key], in2, list(range(CORES)))

    out = np.empty((N_NODES, 64), dtype=np.float32)
    for c in range(CORES):
        upd = np.asarray(res2.results[c]["upd"]).astype(np.float32)
        lo = c * NPC
        for h in range(2):
            loc = colloc[c, h]
            real = loc < NPC
            vals = upd[64 * h:64 * h + 64, :].T
            out[lo + loc[real]] = vals[real]
    return out


# revision 72
# speedup vs baseline: 3.5617x; 1.0012x over previous
"""GNN message-passing layer on 8 trn2 NeuronCores.

Math: messages = relu(x_src@W1 + x_tgt@W2 + b); agg = mean over target;
out = relu(concat(x, agg) @ W_upd + bu).

Plan (target-sharded; host does index work, the A-row gather, and constant
prep only):
  L1 (device): per-core node shard -> A^T=(x@W1)^T and negB'^T=-(x@W2+b)^T
      in one K=66 matmul (ones-row folds the bias), bf16.
  Host: sorts edges by target, nodes by in-degree descending; builds the
      "plane" stream: plane j = the j-th edge slot of every node with
      degree > j, so every slice the device touches is packed.  Gathers A
      rows into that stream (pure data movement; ~2/3 bf16 for DVE 2x
      throughput, 1/3 fp8 to cut DMA bytes).
  L2 (device): Y = max(G, -B') on DVE (relu(G+B') = max(G,-B') + B'; the
      +B' is folded into the PSUM init cnt_corr*B', which also cancels pad
      slots exactly), then PE identity-matmul injection into a PSUM f32
      accumulator (1024-node-column chunks).  agg = acc * (1/count) (ACT
      copy + DVE mult), then the update MLP + relu runs per chunk,
      software-pipelined one chunk behind.

The plane schedule is baked into the NEFF at kernel() time from the actual
edge_index (one SPMD schedule = max across cores; pad slots absorb skew).
"""

import numpy as np
import ml_dtypes

import concourse.bacc as bacc
import concourse.mybir as mybir
import concourse.tile as tile
from concourse.bass_utils import run_bass_kernel_spmd
from concourse.masks import make_identity

N_NODES = 100000
N_EDGES = 1600000
CORES = 8
NPC = N_NODES // CORES          # 12500 nodes per core
NHALF = 6272                    # per-half columns (2*6272 = 12544 >= 12500)
NPAD1 = 12800                   # L1 padded cols (25 x 512)
CHUNKN = 1024                   # node-columns per PSUM accumulation chunk
SEG = 512                       # segment width
DTILE = 8192                    # stream DMA tile width (elements)
# slab assignment pattern: (dtype, path) per slab ordinal mod len(pattern).
# dtype: 0=bf16, 1=fp8.  path: 0 = max(G,-B') on DVE; 1 = relu via PE-inject
# + ACT (PSUM staging), which frees DVE at the cost of PE/ACT work.
SLAB_PATTERN = ((0, 0), (0, 0), (1, 0), (0, 0), (0, 0))
HAS_RELU_PATH = any(p == 1 for _, p in SLAB_PATTERN)

bf16 = mybir.dt.bfloat16
f32 = mybir.dt.float32
fp8 = mybir.dt.float8e4
BF = ml_dtypes.bfloat16
F8 = ml_dtypes.float8_e4m3
# pad value: max(PAD, -B') must equal -B' for any B' (|B'| < 4), and stay
# finite in fp8e4m3 (max 240)
PAD_G = np.float32(-240.0)

_cache = {}


def _plane_schedule(K):
    """Shared host/device schedule with two streams (0=bf16, 1=fp8).

    Returns (chunks, segs, SH) with SH = [SH_bf16, SH_fp8].
    chunks: list of (a, b, slabs, parts) per node-column chunk.
      slabs: (dt, off) - one full-width pair slab ([s x (planeA ws|planeB ws)],
        ws = min(SEG, b-a)) at offset off of stream dt.
      parts: (s, ws, dt, off) - partial-pair segment pair at offset off
        (planeA at off, planeB at off+ws), covering columns [a+s, a+s+ws).
    segs: flat (plane_j, col0, ws, dt, off) for the host gather.
    Slabs never straddle DTILE boundaries; alignment gaps are pad slots no
    compute op reads."""
    npair = len(K) // 2
    chunks = []
    segs = []
    cur = [0, 0]
    sli = 0  # global slab ordinal for dtype assignment
    a = 0

    def align(dt, need):
        if cur[dt] // DTILE != (cur[dt] + need - 1) // DTILE:
            cur[dt] = ((cur[dt] // DTILE) + 1) * DTILE

    while a < NHALF:
        b = min(a + CHUNKN, NHALF)
        w_ch = b - a
        ws_f = min(SEG, w_ch)
        n_s = (w_ch + ws_f - 1) // ws_f
        slab = 2 * w_ch
        full = [p for p in range(npair) if K[2 * p] >= b]
        part = [p for p in range(npair) if a < K[2 * p] < b]
        parts = []
        for p in part:
            w = K[2 * p] - a
            s = 0
            while s < w:
                ws = min(SEG, w - s)
                align(1, 2 * ws)
                off = cur[1]
                parts.append((s, ws, 1, off))
                segs.append((2 * p, a + s, ws, 1, off, 0))
                segs.append((2 * p + 1, a + s, ws, 1, off + ws, 0))
                cur[1] += 2 * ws
                s += ws
        slabs = []
        for p in full:
            dt, path = SLAB_PATTERN[sli % len(SLAB_PATTERN)]
            sli += 1
            align(dt, slab)
            off = cur[dt]
            for si in range(n_s):
                o = off + si * 2 * ws_f
                segs.append((2 * p, a + si * ws_f, ws_f, dt, o, path))
                segs.append((2 * p + 1, a + si * ws_f, ws_f, dt, o + ws_f,
                             path))
            slabs.append((dt, off, path))
            cur[dt] += slab
        chunks.append((a, b, slabs, parts))
        a = b
    SH = [((c + DTILE - 1) // DTILE) * DTILE for c in cur]
    return chunks, segs, SH


def _build_l1():
    nc = bacc.Bacc("TRN2", debug=False, num_devices=CORES)
    xt65 = nc.dram_tensor("xt65", [66, NPAD1], bf16, kind="ExternalInput")
    wab = nc.dram_tensor("wab", [66, 128], bf16, kind="ExternalInput")
    ab = nc.dram_tensor("ab", [128, NPAD1], bf16, kind="ExternalOutput")

    QW = 2560  # 5 tiles of 512 per DMA piece
    with tile.TileContext(nc) as tc:
        with (
            tc.tile_pool(name="big", bufs=1) as big,
            tc.tile_pool(name="psum", bufs=4, space="PSUM") as psum,
        ):
            wt = big.tile([66, 128], bf16)
            xt = big.tile([66, NPAD1], bf16)
            abt = big.tile([128, NPAD1], bf16)
            nc.sync.dma_start(out=wt[:], in_=wab[:, :])
            for q in range(NPAD1 // QW):
                qs = slice(q * QW, (q + 1) * QW)
                nc.sync.dma_start(out=xt[:, qs], in_=xt65[:, qs])
            for c in range(NPAD1 // 512):
                sl = slice(c * 512, (c + 1) * 512)
                pt = psum.tile([128, 512], f32)
                nc.tensor.matmul(out=pt[:], lhsT=wt[:], rhs=xt[:, sl],
                                 start=True, stop=True)
                if c % 2 == 0:
                    nc.vector.tensor_copy(out=abt[:, sl], in_=pt[:])
                else:
                    nc.scalar.activation(
                        out=abt[:, sl], in_=pt[:],
                        func=mybir.ActivationFunctionType.Copy)
                if (c + 1) % 5 == 0:
                    qs = slice((c + 1) * 512 - QW, (c + 1) * 512)
                    nc.sync.dma_start(out=ab[:, qs], in_=abt[:, qs])
    nc.compile()
    return nc


def _build_l2(K):
    chunks, _segs, SH = _plane_schedule(K)
    nc = bacc.Bacc("TRN2", debug=False, num_devices=CORES)
    g16 = nc.dram_tensor("g16", [128, SH[0]], bf16, kind="ExternalInput")
    g8 = nc.dram_tensor("g8", [128, SH[1]], fp8, kind="ExternalInput")
    nb = nc.dram_tensor("nb", [128, NHALF], bf16, kind="ExternalInput")
    ic = nc.dram_tensor("ic", [128, NHALF], bf16, kind="ExternalInput")
    ini = nc.dram_tensor("ini", [128, NHALF], bf16, kind="ExternalInput")
    xu = nc.dram_tensor("xu", [128, NHALF], bf16, kind="ExternalInput")
    wu = nc.dram_tensor("wu", [128, 128], bf16, kind="ExternalInput")
    bu = nc.dram_tensor("bu", [64, 1], f32, kind="ExternalInput")
    upd = nc.dram_tensor("upd", [128, NHALF], bf16, kind="ExternalOutput")

    amax = mybir.AluOpType.max
    mult = mybir.AluOpType.mult
    gdram = (g16, g8)
    gdt = (bf16, fp8)
    ntile = (SH[0] // DTILE, SH[1] // DTILE)

    with tile.TileContext(nc) as tc:
        with (
            tc.tile_pool(name="persist", bufs=1) as per,
            tc.tile_pool(name="st16", bufs=3) as st16p,
            tc.tile_pool(name="st8", bufs=3) as st8p,
            tc.tile_pool(name="ybuf", bufs=4) as ybuf,
            tc.tile_pool(name="ypart", bufs=6) as ypart,
            tc.tile_pool(name="abuf", bufs=3) as abuf,
            tc.tile_pool(name="rbuf", bufs=4) as rbuf,
            tc.tile_pool(name="obuf", bufs=3) as obuf,
            tc.tile_pool(name="acc", bufs=2, space="PSUM") as accp,
            tc.tile_pool(name="upsum", bufs=2, space="PSUM") as upsum,
            tc.tile_pool(name="stage", bufs=2, space="PSUM") as stagep,
        ):
            nb_t = per.tile([128, NHALF], bf16)
            ic_t = per.tile([128, NHALF], bf16)
            ini_t = per.tile([128, NHALF], bf16)
            xu_t = per.tile([128, NHALF], bf16)
            wu_t = per.tile([128, 128], bf16)
            bu_t = per.tile([64, 1], f32)
            agg_t = per.tile([128, NHALF], bf16)
            nbp_t = per.tile([128, NHALF], bf16)
            ident = per.tile([128, 128], bf16)
            ident8 = per.tile([128, 128], fp8)
            nc.scalar.dma_start(out=wu_t[:], in_=wu[:, :])
            nc.scalar.dma_start(out=bu_t[:], in_=bu[:, :])
            make_identity(nc, ident[:])
            if HAS_RELU_PATH:
                make_identity(nc, ident8[:])

            st_tiles = [{}, {}]
            stp = (st16p, st8p)

            def stile(dt, i):
                cachebin = st_tiles[dt]
                if i not in cachebin:
                    t = stp[dt].tile([128, DTILE], gdt[dt], tag="st")
                    h = DTILE // 2
                    nc.sync.dma_start(
                        out=t[:, 0:h],
                        in_=gdram[dt][:, i * DTILE:i * DTILE + h])
                    nc.sync.dma_start(
                        out=t[:, h:],
                        in_=gdram[dt][:, i * DTILE + h:(i + 1) * DTILE])
                    cachebin[i] = t
                return cachebin[i]

            def prologue(ci):
                a, b, _s, _p = chunks[ci]
                nc.sync.dma_start(out=nb_t[:, a:b], in_=nb[:, a:b])
                nc.sync.dma_start(out=ini_t[:, a:b], in_=ini[:, a:b])
                if HAS_RELU_PATH:
                    nc.vector.tensor_scalar_mul(out=nbp_t[:, a:b],
                                                in0=nb_t[:, a:b],
                                                scalar1=-1.0)

            def finish(a, b, acc_t):
                # per-tile: SBUF copy of acc (ACT), agg = copy * (1/count)
                # (DVE), update MLP (PE), relu+bias (ACT), out DMA
                t0 = a
                qi = 0
                while t0 < b:
                    w = min(SEG, b - t0)
                    sl = slice(t0, t0 + w)
                    at = abuf.tile([128, SEG], bf16, tag="at")
                    nc.scalar.activation(
                        out=at[:, 0:w], in_=acc_t[:, t0 - a:t0 - a + w],
                        func=mybir.ActivationFunctionType.Copy)
                    nc.vector.tensor_tensor(out=agg_t[:, sl], in0=at[:, 0:w],
                                            in1=ic_t[:, sl], op=mult)
                    for h in range(2):
                        ps = slice(64 * h, 64 * h + 64)
                        pt = upsum.tile([64, SEG], f32, tag=f"pt{h}")
                        nc.tensor.matmul(out=pt[:, 0:w], lhsT=wu_t[ps, 0:64],
                                         rhs=xu_t[ps, sl],
                                         start=True, stop=False)
                        nc.tensor.matmul(out=pt[:, 0:w],
                                         lhsT=wu_t[ps, 64:128],
                                         rhs=agg_t[ps, sl],
                                         start=False, stop=True)
                        ot = obuf.tile([64, SEG], bf16, tag=f"ot{h}")
                        nc.scalar.activation(
                            out=ot[:, 0:w], in_=pt[:, 0:w],
                            func=mybir.ActivationFunctionType.Relu,
                            bias=bu_t[:])
                        q = (nc.sync, nc.scalar)[qi % 2]
                        qi += 1
                        q.dma_start(out=upd[ps, sl], in_=ot[:, 0:w])
                    t0 += w

            prologue(0)
            prev = None
            for ci, (a, b, slabs, parts) in enumerate(chunks):
                w_ch = b - a
                ws_f = min(SEG, w_ch)
                n_s = (w_ch + ws_f - 1) // ws_f
                slab = 2 * w_ch
                acc_t = accp.tile([128, CHUNKN], f32, tag="acc")
                # ISA: matmul rhs <= [128, 512]
                for s0 in range(0, w_ch, SEG):
                    w = min(SEG, w_ch - s0)
                    nc.tensor.matmul(out=acc_t[:, s0:s0 + w], lhsT=ident[:],
                                     rhs=ini_t[:, a + s0:a + s0 + w],
                                     start=True, stop=False)
                if ci + 1 < len(chunks):
                    prologue(ci + 1)
                n_inj = 2 * n_s * len(slabs) + 2 * len(parts)
                inj = 0
                pend = None

                def inject(rhs_ap, s0, ws, last):
                    nc.tensor.matmul(out=acc_t[:, s0:s0 + ws], lhsT=ident[:],
                                     rhs=rhs_ap, start=False, stop=last)

                for (s, ws, dt, off) in parts:
                    st = stile(dt, off // DTILE)
                    la = off % DTILE
                    cols = slice(a + s, a + s + ws)
                    yt = ypart.tile([128, 2 * SEG], bf16, tag="yp")
                    nbb = nb_t[:, cols].unsqueeze(1).to_broadcast([128, 2, ws])
                    nc.vector.tensor_tensor(out=yt[:, 0:2 * ws],
                                            in0=st[:, la:la + 2 * ws],
                                            in1=nbb, op=amax)
                    inj += 2
                    inject(yt[:, 0:ws], s, ws, False)
                    inject(yt[:, ws:2 * ws], s, ws, inj == n_inj)
                # -B' pattern matching one slab's [s x (A|B)] layout
                base = nb_t[:, a:b]
                if n_s > 1:
                    v = base.rearrange("p (s c) -> p s c", s=n_s)
                    v = v.unsqueeze(2).to_broadcast([128, n_s, 2, ws_f])
                else:
                    v = base.unsqueeze(1).to_broadcast([128, 2, w_ch])
                for si_, (dt, off, path) in enumerate(slabs):
                    ti = off // DTILE
                    la = off % DTILE
                    st = stile(dt, ti)
                    if ti + 1 < ntile[dt]:
                        stile(dt, ti + 1)  # prefetch
                    if si_ == 2 and prev is not None:
                        # software pipeline: previous chunk's finish behind
                        # this chunk's first slabs
                        finish(*prev)
                        prev = None
                    if path == 1:
                        # relu path: stage g + B' in PSUM (PE), relu on ACT,
                        # re-inject one unit behind so PE never waits on ACT.
                        for si in range(n_s):
                            for pl in range(2):
                                lo = la + si * 2 * ws_f + pl * ws_f
                                cs = slice(a + si * ws_f, a + (si + 1) * ws_f)
                                sg = stagep.tile([128, SEG], f32, tag="sg")
                                nc.tensor.matmul(
                                    out=sg[:, 0:ws_f], lhsT=ident8[:],
                                    rhs=st[:, lo:lo + ws_f],
                                    start=True, stop=False)
                                nc.tensor.matmul(
                                    out=sg[:, 0:ws_f], lhsT=ident[:],
                                    rhs=nbp_t[:, cs], start=False, stop=True)
                                yo = rbuf.tile([128, SEG], bf16, tag="yo")
                                nc.scalar.activation(
                                    out=yo[:, 0:ws_f], in_=sg[:, 0:ws_f],
                                    func=mybir.ActivationFunctionType.Relu)
                                if pend is not None:
                                    inj += 1
                                    inject(pend[0][:, 0:pend[2]], pend[1],
                                           pend[2], inj == n_inj)
                                pend = (yo, si * ws_f, ws_f)
                        continue
                    yt = ybuf.tile([128, 2 * CHUNKN], bf16, tag="yt")
                    nc.vector.tensor_tensor(out=yt[:, 0:slab],
                                            in0=st[:, la:la + slab],
                                            in1=v, op=amax)
                    for si in range(n_s):
                        lo = si * 2 * ws_f
                        inj += 2
                        inject(yt[:, lo:lo + ws_f], si * ws_f, ws_f, False)
                        inject(yt[:, lo + ws_f:lo + 2 * ws_f],
                               si * ws_f, ws_f, inj == n_inj)
                if pend is not None:
                    inj += 1
                    inject(pend[0][:, 0:pend[2]], pend[1], pend[2],
                           inj == n_inj)
                    pend = None
                if prev is not None:
                    finish(*prev)
                # update-phase inputs, consumed by finish() one chunk later
                nc.scalar.dma_start(out=ic_t[:, a:b], in_=ic[:, a:b])
                nc.scalar.dma_start(out=xu_t[:, a:b], in_=xu[:, a:b])
                prev = (a, b, acc_t)
            finish(*prev)
    nc.compile()
    return nc


def kernel(x, edge_index, W_msg, b_msg, W_upd, b_upd):
    x = np.asarray(x, dtype=np.float32)
    src = np.asarray(edge_index[0], dtype=np.int64)
    tgt = np.asarray(edge_index[1], dtype=np.int64)
    W_msg = np.asarray(W_msg, dtype=np.float32)
    b_msg = np.asarray(b_msg, dtype=np.float32)
    W_upd = np.asarray(W_upd, dtype=np.float32)
    b_upd = np.asarray(b_upd, dtype=np.float32)

    # ---------------- L1 ----------------
    if "l1" not in _cache:
        _cache["l1"] = _build_l1()
    wab = np.zeros((66, 128), dtype=np.float32)
    wab[:64, :64] = W_msg[:64]
    wab[:64, 64:] = -W_msg[64:]
    wab[64, 64:] = -b_msg
    wab = wab.astype(BF)
    xb = x.astype(BF)
    in1 = []
    for c in range(CORES):
        xt65 = np.zeros((66, NPAD1), dtype=BF)
        xt65[:64, :NPC] = xb[c * NPC:(c + 1) * NPC].T
        xt65[64, :] = np.float32(1.0)
        in1.append({"xt65": xt65, "wab": wab})
    res1 = run_bass_kernel_spmd(_cache["l1"], in1, list(range(CORES)))
    A_T = np.concatenate([np.asarray(r["ab"])[0:64, :NPC]
                          for r in res1.results], axis=1)
    negB_T = np.concatenate([np.asarray(r["ab"])[64:128, :NPC]
                             for r in res1.results], axis=1)

    # ---------------- host: plane schedule ----------------
    counts = np.bincount(tgt, minlength=N_NODES).astype(np.int64)
    order = np.argsort(tgt, kind="stable")
    cum = np.zeros(N_NODES + 1, dtype=np.int64)
    np.cumsum(counts, out=cum[1:])

    colloc = np.empty((CORES, 2, NHALF), dtype=np.int64)
    colnode = np.empty((CORES, 2, NHALF), dtype=np.int64)
    colcnt = np.zeros((CORES, 2, NHALF), dtype=np.int64)
    for c in range(CORES):
        lo = c * NPC
        cnt_loc = np.zeros(2 * NHALF, dtype=np.int64)
        cnt_loc[:NPC] = counts[lo:lo + NPC]
        rank = np.argsort(-cnt_loc, kind="stable")
        for h in range(2):
            nodes = rank[h::2]
            colloc[c, h] = nodes
            colnode[c, h] = np.minimum(nodes, NPC - 1) + lo
            colcnt[c, h] = cnt_loc[nodes]

    tmax = int(colcnt.max())
    tmax += tmax % 2
    K = np.zeros(tmax, dtype=np.int64)
    for j in range(tmax):
        K[j] = int((colcnt > j).sum(axis=2).max())
    for p in range(tmax // 2):
        K[2 * p + 1] = K[2 * p]
    K = [int(k) for k in K if k > 0]
    if len(K) % 2:
        K.append(K[-1])

    key = ("l2", tuple(K))
    if key not in _cache:
        _cache[key] = _build_l2(K)
    chunks, segs, SH = _plane_schedule(K)

    # cnt_corr: planes covering each column via the max-path only (the
    # relu path needs no correction; its pads contribute exactly zero)
    cnt_corr = np.zeros(NHALF, dtype=np.float32)
    for (_jj, col0, ws, _dt, _off, path) in segs:
        if path == 0:
            cnt_corr[col0:col0 + ws] += 1

    A_np = np.asarray(A_T)
    A8 = A_np.astype(F8)
    nB_np = np.asarray(negB_T)
    xbT = np.ascontiguousarray(xb.T)
    wu = np.zeros((128, 128), dtype=np.float32)
    wu[:64, :64] = W_upd[:64]
    wu[:64, 64:] = W_upd[64:]
    wu[64:] = wu[:64]
    wu = wu.astype(BF)
    bu = b_upd.reshape(64, 1).astype(np.float32)

    in2 = []
    for c in range(CORES):
        G16 = np.full((128, SH[0]), PAD_G, dtype=BF)
        G8 = np.full((128, SH[1]), PAD_G, dtype=F8)
        for h in range(2):
            nodes = colnode[c, h]
            ncnt = colcnt[c, h]
            starts = cum[nodes]
            for dt, Gt, At in ((0, G16, A_np), (1, G8, A8)):
                srcflat = np.full(SH[dt], -1, dtype=np.int64)
                for (jj, col0, ws, sdt, off, _path) in segs:
                    if sdt != dt:
                        continue
                    csl = slice(col0, col0 + ws)
                    valid = ncnt[csl] > jj
                    srcflat[off:off + ws] = np.where(
                        valid, starts[csl] + jj, -1)
                have = srcflat >= 0
                idx = src[order[srcflat[have]]]
                Gh = np.full((64, SH[dt]), PAD_G, dtype=Gt.dtype)
                Gh[:, have] = At[:, idx]
                Gt[64 * h:64 * h + 64] = Gh
        nb2 = np.empty((128, NHALF), dtype=BF)
        ic2 = np.empty((128, NHALF), dtype=BF)
        ini2 = np.empty((128, NHALF), dtype=BF)
        xu2 = np.empty((128, NHALF), dtype=BF)
        for h in range(2):
            r = slice(64 * h, 64 * h + 64)
            nbh = nB_np[:, colnode[c, h]]
            nb2[r] = nbh
            xu2[r] = xbT[:, colnode[c, h]]
            ic2[r] = (1.0 / np.maximum(colcnt[c, h], 1)).astype(BF)[None, :]
            # ini = cnt_corr * B' = (-cnt_corr) * (-B')
            ini2[r] = ((-cnt_corr[None, :]) *
                       nbh.astype(np.float32)).astype(BF)
        in2.append({"g16": G16, "g8": G8, "nb": nb2, "ic": ic2, "ini": ini2,
                    "xu": xu2, "wu": wu, "bu": bu})

    res2 = run_bass_kernel_spmd(_cache[key], in2, list(range(CORES)))

    out = np.empty((N_NODES, 64), dtype=np.float32)
    for c in range(CORES):
        upd = np.asarray(res2.results[c]["upd"]).astype(np.float32)
        lo = c * NPC
        for h in range(2):
            loc = colloc[c, h]
            real = loc < NPC
            vals = upd[64 * h:64 * h + 64, :].T
            out[lo + loc[real]] = vals[real]
    return out


# revision 77
# speedup vs baseline: 3.5928x; 1.0087x over previous
"""GNN message-passing layer on 8 trn2 NeuronCores.

Math: messages = relu(x_src@W1 + x_tgt@W2 + b); agg = mean over target;
out = relu(concat(x, agg) @ W_upd + bu).

Plan (target-sharded; host does index work, the A-row gather, and constant
prep only):
  L1 (device): per-core node shard -> A^T=(x@W1)^T and negB'^T=-(x@W2+b)^T
      in one K=66 matmul (ones-row folds the bias), bf16.
  Host: sorts edges by target, nodes by in-degree descending; builds the
      "plane" stream: plane j = the j-th edge slot of every node with
      degree > j, so every slice the device touches is packed.  Gathers A
      rows into that stream (pure data movement; ~2/3 bf16 for DVE 2x
      throughput, 1/3 fp8 to cut DMA bytes).
  L2 (device): Y = max(G, -B') on DVE (relu(G+B') = max(G,-B') + B'; the
      +B' is folded into the PSUM init cnt_corr*B', which also cancels pad
      slots exactly), then PE identity-matmul injection into a PSUM f32
      accumulator (1024-node-column chunks).  agg = acc * (1/count) (ACT
      copy + DVE mult), then the update MLP + relu runs per chunk,
      software-pipelined one chunk behind.

The plane schedule is baked into the NEFF at kernel() time from the actual
edge_index (one SPMD schedule = max across cores; pad slots absorb skew).
"""

import numpy as np
import ml_dtypes

import concourse.bacc as bacc
import concourse.mybir as mybir
import concourse.tile as tile
from concourse.bass_utils import run_bass_kernel_spmd
from concourse.masks import make_identity

N_NODES = 100000
N_EDGES = 1600000
CORES = 8
NPC = N_NODES // CORES          # 12500 nodes per core
NHALF = 6272                    # per-half columns (2*6272 = 12544 >= 12500)
NPAD1 = 12800                   # L1 padded cols (25 x 512)
CHUNKN = 1024                   # node-columns per PSUM accumulation chunk
SEG = 512                       # segment width
DTILE = 8192                    # stream DMA tile width (elements)
# slab assignment pattern: (dtype, path) per slab ordinal mod len(pattern).
# dtype: 0=bf16, 1=fp8.  path: 0 = max(G,-B') on DVE; 1 = relu via PE-inject
# + ACT (PSUM staging), which frees DVE at the cost of PE/ACT work.
SLAB_PATTERN = ((0, 0), (0, 0), (1, 0), (0, 0), (0, 0))
HAS_RELU_PATH = any(p == 1 for _, p in SLAB_PATTERN)

bf16 = mybir.dt.bfloat16
f32 = mybir.dt.float32
fp8 = mybir.dt.float8e4
BF = ml_dtypes.bfloat16
F8 = ml_dtypes.float8_e4m3
# pad value: max(PAD, -B') must equal -B' for any B' (|B'| < 4), and stay
# finite in fp8e4m3 (max 240)
PAD_G = np.float32(-240.0)

_cache = {}


def _plane_schedule(K):
    """Shared host/device schedule with two streams (0=bf16, 1=fp8).

    Returns (chunks, segs, SH) with SH = [SH_bf16, SH_fp8].
    chunks: list of (a, b, slabs, parts) per node-column chunk.
      slabs: (dt, off) - one full-width pair slab ([s x (planeA ws|planeB ws)],
        ws = min(SEG, b-a)) at offset off of stream dt.
      parts: (s, ws, dt, off) - partial-pair segment pair at offset off
        (planeA at off, planeB at off+ws), covering columns [a+s, a+s+ws).
    segs: flat (plane_j, col0, ws, dt, off) for the host gather.
    Slabs never straddle DTILE boundaries; alignment gaps are pad slots no
    compute op reads."""
    npair = len(K) // 2
    chunks = []
    segs = []
    cur = [0, 0]
    sli = 0  # global slab ordinal for dtype assignment
    a = 0

    def align(dt, need):
        if cur[dt] // DTILE != (cur[dt] + need - 1) // DTILE:
            cur[dt] = ((cur[dt] // DTILE) + 1) * DTILE

    while a < NHALF:
        b = min(a + CHUNKN, NHALF)
        w_ch = b - a
        ws_f = min(SEG, w_ch)
        n_s = (w_ch + ws_f - 1) // ws_f
        slab = 2 * w_ch
        full = [p for p in range(npair) if K[2 * p] >= b]
        part = [p for p in range(npair) if a < K[2 * p] < b]
        parts = []
        for p in part:
            w = K[2 * p] - a
            s = 0
            while s < w:
                ws = min(SEG, w - s)
                align(1, 2 * ws)
                off = cur[1]
                parts.append((s, ws, 1, off))
                segs.append((2 * p, a + s, ws, 1, off, 0))
                segs.append((2 * p + 1, a + s, ws, 1, off + ws, 0))
                cur[1] += 2 * ws
                s += ws
        slabs = []
        for p in full:
            dt, path = SLAB_PATTERN[sli % len(SLAB_PATTERN)]
            sli += 1
            align(dt, slab)
            off = cur[dt]
            for si in range(n_s):
                o = off + si * 2 * ws_f
                segs.append((2 * p, a + si * ws_f, ws_f, dt, o, path))
                segs.append((2 * p + 1, a + si * ws_f, ws_f, dt, o + ws_f,
                             path))
            slabs.append((dt, off, path))
            cur[dt] += slab
        chunks.append((a, b, slabs, parts))
        a = b
    SH = [((c + DTILE - 1) // DTILE) * DTILE for c in cur]
    return chunks, segs, SH


def _build_l1():
    nc = bacc.Bacc("TRN2", debug=False, num_devices=CORES)
    xt65 = nc.dram_tensor("xt65", [66, NPAD1], bf16, kind="ExternalInput")
    wab = nc.dram_tensor("wab", [66, 128], bf16, kind="ExternalInput")
    ab = nc.dram_tensor("ab", [128, NPAD1], bf16, kind="ExternalOutput")

    QW = 2560  # 5 tiles of 512 per DMA piece
    with tile.TileContext(nc) as tc:
        with (
            tc.tile_pool(name="big", bufs=1) as big,
            tc.tile_pool(name="psum", bufs=4, space="PSUM") as psum,
        ):
            wt = big.tile([66, 128], bf16)
            xt = big.tile([66, NPAD1], bf16)
            abt = big.tile([128, NPAD1], bf16)
            nc.sync.dma_start(out=wt[:], in_=wab[:, :])
            for q in range(NPAD1 // QW):
                qs = slice(q * QW, (q + 1) * QW)
                nc.sync.dma_start(out=xt[:, qs], in_=xt65[:, qs])
            for c in range(NPAD1 // 512):
                sl = slice(c * 512, (c + 1) * 512)
                pt = psum.tile([128, 512], f32)
                nc.tensor.matmul(out=pt[:], lhsT=wt[:], rhs=xt[:, sl],
                                 start=True, stop=True)
                if c % 2 == 0:
                    nc.vector.tensor_copy(out=abt[:, sl], in_=pt[:])
                else:
                    nc.scalar.activation(
                        out=abt[:, sl], in_=pt[:],
                        func=mybir.ActivationFunctionType.Copy)
                if (c + 1) % 5 == 0:
                    qs = slice((c + 1) * 512 - QW, (c + 1) * 512)
                    nc.sync.dma_start(out=ab[:, qs], in_=abt[:, qs])
    nc.compile()
    return nc


def _build_l2(K):
    chunks, _segs, SH = _plane_schedule(K)
    nc = bacc.Bacc("TRN2", debug=False, num_devices=CORES)
    g16 = nc.dram_tensor("g16", [128, SH[0]], bf16, kind="ExternalInput")
    g8 = nc.dram_tensor("g8", [128, SH[1]], fp8, kind="ExternalInput")
    nb = nc.dram_tensor("nb", [128, NHALF], bf16, kind="ExternalInput")
    ic = nc.dram_tensor("ic", [128, NHALF], bf16, kind="ExternalInput")
    ini = nc.dram_tensor("ini", [128, NHALF], bf16, kind="ExternalInput")
    xu = nc.dram_tensor("xu", [128, NHALF], bf16, kind="ExternalInput")
    wu = nc.dram_tensor("wu", [128, 128], bf16, kind="ExternalInput")
    bu = nc.dram_tensor("bu", [64, 1], f32, kind="ExternalInput")
    upd = nc.dram_tensor("upd", [128, NHALF], bf16, kind="ExternalOutput")

    amax = mybir.AluOpType.max
    mult = mybir.AluOpType.mult
    gdram = (g16, g8)
    gdt = (bf16, fp8)
    ntile = (SH[0] // DTILE, SH[1] // DTILE)

    with tile.TileContext(nc) as tc:
        with (
            tc.tile_pool(name="persist", bufs=1) as per,
            tc.tile_pool(name="st16", bufs=3) as st16p,
            tc.tile_pool(name="st8", bufs=3) as st8p,
            tc.tile_pool(name="ybuf", bufs=4) as ybuf,
            tc.tile_pool(name="ypart", bufs=6) as ypart,
            tc.tile_pool(name="abuf", bufs=3) as abuf,
            tc.tile_pool(name="rbuf", bufs=4) as rbuf,
            tc.tile_pool(name="obuf", bufs=3) as obuf,
            tc.tile_pool(name="acc", bufs=2, space="PSUM") as accp,
            tc.tile_pool(name="upsum", bufs=2, space="PSUM") as upsum,
            tc.tile_pool(name="stage", bufs=2, space="PSUM") as stagep,
        ):
            nb_t = per.tile([128, NHALF], bf16)
            ic_t = per.tile([128, NHALF], bf16)
            ini_t = per.tile([128, NHALF], bf16)
            xu_t = per.tile([128, NHALF], bf16)
            wu_t = per.tile([128, 128], bf16)
            bu_t = per.tile([64, 1], f32)
            agg_t = per.tile([128, NHALF], bf16)
            out_t = per.tile([128, NHALF], bf16)
            nbp_t = per.tile([128, NHALF], bf16)
            ident = per.tile([128, 128], bf16)
            ident8 = per.tile([128, 128], fp8)
            nc.scalar.dma_start(out=wu_t[:], in_=wu[:, :])
            nc.scalar.dma_start(out=bu_t[:], in_=bu[:, :])
            make_identity(nc, ident[:])
            if HAS_RELU_PATH:
                make_identity(nc, ident8[:])

            st_tiles = [{}, {}]
            stp = (st16p, st8p)

            def stile(dt, i):
                cachebin = st_tiles[dt]
                if i not in cachebin:
                    t = stp[dt].tile([128, DTILE], gdt[dt], tag="st")
                    h = DTILE // 2
                    nc.sync.dma_start(
                        out=t[:, 0:h],
                        in_=gdram[dt][:, i * DTILE:i * DTILE + h])
                    nc.sync.dma_start(
                        out=t[:, h:],
                        in_=gdram[dt][:, i * DTILE + h:(i + 1) * DTILE])
                    cachebin[i] = t
                return cachebin[i]

            def prologue(ci):
                a, b, _s, _p = chunks[ci]
                nc.sync.dma_start(out=nb_t[:, a:b], in_=nb[:, a:b])
                nc.sync.dma_start(out=ini_t[:, a:b], in_=ini[:, a:b])
                if HAS_RELU_PATH:
                    nc.vector.tensor_scalar_mul(out=nbp_t[:, a:b],
                                                in0=nb_t[:, a:b],
                                                scalar1=-1.0)

            def finish(a, b, acc_t):
                # per-tile: SBUF copy of acc (ACT), agg = copy * (1/count)
                # (DVE), update MLP (PE), relu+bias (ACT) into chunk-wide
                # per-half buffers; one out DMA per half per chunk
                och0 = obuf.tile([64, CHUNKN], bf16, tag="ot0")
                och1 = obuf.tile([64, CHUNKN], bf16, tag="ot1")
                och = (och0, och1)
                t0 = a
                while t0 < b:
                    w = min(SEG, b - t0)
                    sl = slice(t0, t0 + w)
                    at = abuf.tile([128, SEG], bf16, tag="at")
                    nc.scalar.activation(
                        out=at[:, 0:w], in_=acc_t[:, t0 - a:t0 - a + w],
                        func=mybir.ActivationFunctionType.Copy)
                    nc.vector.tensor_tensor(out=agg_t[:, sl], in0=at[:, 0:w],
                                            in1=ic_t[:, sl], op=mult)
                    for h in range(2):
                        ps = slice(64 * h, 64 * h + 64)
                        pt = upsum.tile([64, SEG], f32, tag=f"pt{h}")
                        nc.tensor.matmul(out=pt[:, 0:w], lhsT=wu_t[ps, 0:64],
                                         rhs=xu_t[ps, sl],
                                         start=True, stop=False)
                        nc.tensor.matmul(out=pt[:, 0:w],
                                         lhsT=wu_t[ps, 64:128],
                                         rhs=agg_t[ps, sl],
                                         start=False, stop=True)
                        nc.scalar.activation(
                            out=och[h][:, t0 - a:t0 - a + w], in_=pt[:, 0:w],
                            func=mybir.ActivationFunctionType.Relu,
                            bias=bu_t[:])
                    t0 += w
                for h in range(2):
                    ps = slice(64 * h, 64 * h + 64)
                    q = (nc.sync, nc.scalar)[h]
                    q.dma_start(out=upd[ps, a:b], in_=och[h][:, 0:b - a])

            prologue(0)
            prev = None
            for ci, (a, b, slabs, parts) in enumerate(chunks):
                w_ch = b - a
                ws_f = min(SEG, w_ch)
                n_s = (w_ch + ws_f - 1) // ws_f
                slab = 2 * w_ch
                acc_t = accp.tile([128, CHUNKN], f32, tag="acc")
                # ISA: matmul rhs <= [128, 512]
                for s0 in range(0, w_ch, SEG):
                    w = min(SEG, w_ch - s0)
                    nc.tensor.matmul(out=acc_t[:, s0:s0 + w], lhsT=ident[:],
                                     rhs=ini_t[:, a + s0:a + s0 + w],
                                     start=True, stop=False)
                if ci + 1 < len(chunks):
                    prologue(ci + 1)
                n_inj = 2 * n_s * len(slabs) + 2 * len(parts)
                inj = 0
                pend = None

                def inject(rhs_ap, s0, ws, last):
                    nc.tensor.matmul(out=acc_t[:, s0:s0 + ws], lhsT=ident[:],
                                     rhs=rhs_ap, start=False, stop=last)

                for (s, ws, dt, off) in parts:
                    st = stile(dt, off // DTILE)
                    la = off % DTILE
                    cols = slice(a + s, a + s + ws)
                    yt = ypart.tile([128, 2 * SEG], bf16, tag="yp")
                    nbb = nb_t[:, cols].unsqueeze(1).to_broadcast([128, 2, ws])
                    nc.vector.tensor_tensor(out=yt[:, 0:2 * ws],
                                            in0=st[:, la:la + 2 * ws],
                                            in1=nbb, op=amax)
                    inj += 2
                    inject(yt[:, 0:ws], s, ws, False)
                    inject(yt[:, ws:2 * ws], s, ws, inj == n_inj)
                # -B' pattern matching one slab's [s x (A|B)] layout
                base = nb_t[:, a:b]
                if n_s > 1:
                    v = base.rearrange("p (s c) -> p s c", s=n_s)
                    v = v.unsqueeze(2).to_broadcast([128, n_s, 2, ws_f])
                else:
                    v = base.unsqueeze(1).to_broadcast([128, 2, w_ch])
                for si_, (dt, off, path) in enumerate(slabs):
                    ti = off // DTILE
                    la = off % DTILE
                    st = stile(dt, ti)
                    if ti + 1 < ntile[dt]:
                        stile(dt, ti + 1)  # prefetch
                    if si_ == 2 and prev is not None:
                        # software pipeline: previous chunk's finish behind
                        # this chunk's first slabs
                        finish(*prev)
                        prev = None
                    if path == 1:
                        # relu path: stage g + B' in PSUM (PE), relu on ACT,
                        # re-inject one unit behind so PE never waits on ACT.
                        for si in range(n_s):
                            for pl in range(2):
                                lo = la + si * 2 * ws_f + pl * ws_f
                                cs = slice(a + si * ws_f, a + (si + 1) * ws_f)
                                sg = stagep.tile([128, SEG], f32, tag="sg")
                                nc.tensor.matmul(
                                    out=sg[:, 0:ws_f], lhsT=ident8[:],
                                    rhs=st[:, lo:lo + ws_f],
                                    start=True, stop=False)
                                nc.tensor.matmul(
                                    out=sg[:, 0:ws_f], lhsT=ident[:],
                                    rhs=nbp_t[:, cs], start=False, stop=True)
                                yo = rbuf.tile([128, SEG], bf16, tag="yo")
                                nc.scalar.activation(
                                    out=yo[:, 0:ws_f], in_=sg[:, 0:ws_f],
                                    func=mybir.ActivationFunctionType.Relu)
                                if pend is not None:
                                    inj += 1
                                    inject(pend[0][:, 0:pend[2]], pend[1],
                                           pend[2], inj == n_inj)
                                pend = (yo, si * ws_f, ws_f)
                        continue
                    yt = ybuf.tile([128, 2 * CHUNKN], bf16, tag="yt")
                    nc.vector.tensor_tensor(out=yt[:, 0:slab],
                                            in0=st[:, la:la + slab],
                                            in1=v, op=amax)
                    for si in range(n_s):
                        lo = si * 2 * ws_f
                        inj += 2
                        inject(yt[:, lo:lo + ws_f], si * ws_f, ws_f, False)
                        inject(yt[:, lo + ws_f:lo + 2 * ws_f],
                               si * ws_f, ws_f, inj == n_inj)
                if pend is not None:
                    inj += 1
                    inject(pend[0][:, 0:pend[2]], pend[1], pend[2],
                           inj == n_inj)
                    pend = None
                if prev is not None:
                    finish(*prev)
                # update-phase inputs, consumed by finish() one chunk later
                nc.scalar.dma_start(out=ic_t[:, a:b], in_=ic[:, a:b])
                nc.scalar.dma_start(out=xu_t[:, a:b], in_=xu[:, a:b])
                prev = (a, b, acc_t)
            finish(*prev)
    nc.compile()
    return nc


def kernel(x, edge_index, W_msg, b_msg, W_upd, b_upd):
    x = np.asarray(x, dtype=np.float32)
    src = np.asarray(edge_index[0], dtype=np.int64)
    tgt = np.asarray(edge_index[1], dtype=np.int64)
    W_msg = np.asarray(W_msg, dtype=np.float32)
    b_msg = np.asarray(b_msg, dtype=np.float32)
    W_upd = np.asarray(W_upd, dtype=np.float32)
    b_upd = np.asarray(b_upd, dtype=np.float32)

    # ---------------- L1 ----------------
    if "l1" not in _cache:
        _cache["l1"] = _build_l1()
    wab = np.zeros((66, 128), dtype=np.float32)
    wab[:64, :64] = W_msg[:64]
    wab[:64, 64:] = -W_msg[64:]
    wab[64, 64:] = -b_msg
    wab = wab.astype(BF)
    xb = x.astype(BF)
    in1 = []
    for c in range(CORES):
        xt65 = np.zeros((66, NPAD1), dtype=BF)
        xt65[:64, :NPC] = xb[c * NPC:(c + 1) * NPC].T
        xt65[64, :] = np.float32(1.0)
        in1.append({"xt65": xt65, "wab": wab})
    res1 = run_bass_kernel_spmd(_cache["l1"], in1, list(range(CORES)))
    A_T = np.concatenate([np.asarray(r["ab"])[0:64, :NPC]
                          for r in res1.results], axis=1)
    negB_T = np.concatenate([np.asarray(r["ab"])[64:128, :NPC]
                             for r in res1.results], axis=1)

    # ---------------- host: plane schedule ----------------
    counts = np.bincount(tgt, minlength=N_NODES).astype(np.int64)
    order = np.argsort(tgt, kind="stable")
    cum = np.zeros(N_NODES + 1, dtype=np.int64)
    np.cumsum(counts, out=cum[1:])

    colloc = np.empty((CORES, 2, NHALF), dtype=np.int64)
    colnode = np.empty((CORES, 2, NHALF), dtype=np.int64)
    colcnt = np.zeros((CORES, 2, NHALF), dtype=np.int64)
    for c in range(CORES):
        lo = c * NPC
        cnt_loc = np.zeros(2 * NHALF, dtype=np.int64)
        cnt_loc[:NPC] = counts[lo:lo + NPC]
        rank = np.argsort(-cnt_loc, kind="stable")
        for h in range(2):
            nodes = rank[h::2]
            colloc[c, h] = nodes
            colnode[c, h] = np.minimum(nodes, NPC - 1) + lo
            colcnt[c, h] = cnt_loc[nodes]

    tmax = int(colcnt.max())
    tmax += tmax % 2
    K = np.zeros(tmax, dtype=np.int64)
    for j in range(tmax):
        K[j] = int((colcnt > j).sum(axis=2).max())
    for p in range(tmax // 2):
        K[2 * p + 1] = K[2 * p]
    K = [int(k) for k in K if k > 0]
    if len(K) % 2:
        K.append(K[-1])

    key = ("l2", tuple(K))
    if key not in _cache:
        _cache[key] = _build_l2(K)
    chunks, segs, SH = _plane_schedule(K)

    # cnt_corr: planes covering each column via the max-path only (the
    # relu path needs no correction; its pads contribute exactly zero)
    cnt_corr = np.zeros(NHALF, dtype=np.float32)
    for (_jj, col0, ws, _dt, _off, path) in segs:
        if path == 0:
            cnt_corr[col0:col0 + ws] += 1

    A_np = np.asarray(A_T)
    A8 = A_np.astype(F8)
    nB_np = np.asarray(negB_T)
    xbT = np.ascontiguousarray(xb.T)
    wu = np.zeros((128, 128), dtype=np.float32)
    wu[:64, :64] = W_upd[:64]
    wu[:64, 64:] = W_upd[64:]
    wu[64:] = wu[:64]
    wu = wu.astype(BF)
    bu = b_upd.reshape(64, 1).astype(np.float32)

    in2 = []
    for c in range(CORES):
        G16 = np.full((128, SH[0]), PAD_G, dtype=BF)
        G8 = np.full((128, SH[1]), PAD_G, dtype=F8)
        for h in range(2):
            nodes = colnode[c, h]
            ncnt = colcnt[c, h]
            starts = cum[nodes]
            for dt, Gt, At in ((0, G16, A_np), (1, G8, A8)):
                srcflat = np.full(SH[dt], -1, dtype=np.int64)
                for (jj, col0, ws, sdt, off, _path) in segs:
                    if sdt != dt:
                        continue
                    csl = slice(col0, col0 + ws)
                    valid = ncnt[csl] > jj
                    srcflat[off:off + ws] = np.where(
                        valid, starts[csl] + jj, -1)
                have = srcflat >= 0
                idx = src[order[srcflat[have]]]
                Gh = np.full((64, SH[dt]), PAD_G, dtype=Gt.dtype)
                Gh[:, have] = At[:, idx]
                Gt[64 * h:64 * h + 64] = Gh
        nb2 = np.empty((128, NHALF), dtype=BF)
        ic2 = np.empty((128, NHALF), dtype=BF)
        ini2 = np.empty((128, NHALF), dtype=BF)
        xu2 = np.empty((128, NHALF), dtype=BF)
        for h in range(2):
            r = slice(64 * h, 64 * h + 64)
            nbh = nB_np[:, colnode[c, h]]
            nb2[r] = nbh
            xu2[r] = xbT[:, colnode[c, h]]
            ic2[r] = (1.0 / np.maximum(colcnt[c, h], 1)).astype(BF)[None, :]
            # ini = cnt_corr * B' = (-cnt_corr) * (-B')
            ini2[r] = ((-cnt_corr[None, :]) *
                       nbh.astype(np.float32)).astype(BF)
        in2.append({"g16": G16, "g8": G8, "nb": nb2, "ic": ic2, "ini": ini2,
                    "xu": xu2, "wu": wu, "bu": bu})

    res2 = run_bass_kernel_spmd(_cache[key], in2, list(range(CORES)))

    out = np.empty((N_NODES, 64), dtype=np.float32)
    for c in range(CORES):
        upd = np.asarray(res2.results[c]["upd"]).astype(np.float32)
        lo = c * NPC
        for h in range(2):
            loc = colloc[c, h]
            real = loc < NPC
            vals = upd[64 * h:64 * h + 64, :].T
            out[lo + loc[real]] = vals[real]
    return out


# revision 78
# speedup vs baseline: 3.6004x; 1.0021x over previous
"""GNN message-passing layer on 8 trn2 NeuronCores.

Math: messages = relu(x_src@W1 + x_tgt@W2 + b); agg = mean over target;
out = relu(concat(x, agg) @ W_upd + bu).

Plan (target-sharded; host does index work, the A-row gather, and constant
prep only):
  L1 (device): per-core node shard -> A^T=(x@W1)^T and negB'^T=-(x@W2+b)^T
      in one K=66 matmul (ones-row folds the bias), bf16.
  Host: sorts edges by target, nodes by in-degree descending; builds the
      "plane" stream: plane j = the j-th edge slot of every node with
      degree > j, so every slice the device touches is packed.  Gathers A
      rows into that stream (pure data movement; ~2/3 bf16 for DVE 2x
      throughput, 1/3 fp8 to cut DMA bytes).
  L2 (device): Y = max(G, -B') on DVE (relu(G+B') = max(G,-B') + B'; the
      +B' is folded into the PSUM init cnt_corr*B', which also cancels pad
      slots exactly), then PE identity-matmul injection into a PSUM f32
      accumulator (1024-node-column chunks).  agg = acc * (1/count) (ACT
      copy + DVE mult), then the update MLP + relu runs per chunk,
      software-pipelined one chunk behind.

The plane schedule is baked into the NEFF at kernel() time from the actual
edge_index (one SPMD schedule = max across cores; pad slots absorb skew).
"""

import numpy as np
import ml_dtypes

import concourse.bacc as bacc
import concourse.mybir as mybir
import concourse.tile as tile
from concourse.bass_utils import run_bass_kernel_spmd
from concourse.masks import make_identity

N_NODES = 100000
N_EDGES = 1600000
CORES = 8
NPC = N_NODES // CORES          # 12500 nodes per core
NHALF = 6272                    # per-half columns (2*6272 = 12544 >= 12500)
NPAD1 = 12800                   # L1 padded cols (25 x 512)
CHUNKN = 1024                   # node-columns per PSUM accumulation chunk
SEG = 512                       # segment width
DTILE = 8192                    # stream DMA tile width (elements)
# slab assignment pattern: (dtype, path) per slab ordinal mod len(pattern).
# dtype: 0=bf16, 1=fp8.  path: 0 = max(G,-B') on DVE; 1 = relu via PE-inject
# + ACT (PSUM staging), which frees DVE at the cost of PE/ACT work.
SLAB_PATTERN = ((1, 0), (0, 0), (0, 0), (0, 0), (0, 0))
HAS_RELU_PATH = any(p == 1 for _, p in SLAB_PATTERN)

bf16 = mybir.dt.bfloat16
f32 = mybir.dt.float32
fp8 = mybir.dt.float8e4
BF = ml_dtypes.bfloat16
F8 = ml_dtypes.float8_e4m3
# pad value: max(PAD, -B') must equal -B' for any B' (|B'| < 4), and stay
# finite in fp8e4m3 (max 240)
PAD_G = np.float32(-240.0)

_cache = {}


def _plane_schedule(K):
    """Shared host/device schedule with two streams (0=bf16, 1=fp8).

    Returns (chunks, segs, SH) with SH = [SH_bf16, SH_fp8].
    chunks: list of (a, b, slabs, parts) per node-column chunk.
      slabs: (dt, off) - one full-width pair slab ([s x (planeA ws|planeB ws)],
        ws = min(SEG, b-a)) at offset off of stream dt.
      parts: (s, ws, dt, off) - partial-pair segment pair at offset off
        (planeA at off, planeB at off+ws), covering columns [a+s, a+s+ws).
    segs: flat (plane_j, col0, ws, dt, off) for the host gather.
    Slabs never straddle DTILE boundaries; alignment gaps are pad slots no
    compute op reads."""
    npair = len(K) // 2
    chunks = []
    segs = []
    cur = [0, 0]
    sli = 0  # global slab ordinal for dtype assignment
    a = 0

    def align(dt, need):
        if cur[dt] // DTILE != (cur[dt] + need - 1) // DTILE:
            cur[dt] = ((cur[dt] // DTILE) + 1) * DTILE

    while a < NHALF:
        b = min(a + CHUNKN, NHALF)
        w_ch = b - a
        ws_f = min(SEG, w_ch)
        n_s = (w_ch + ws_f - 1) // ws_f
        slab = 2 * w_ch
        full = [p for p in range(npair) if K[2 * p] >= b]
        part = [p for p in range(npair) if a < K[2 * p] < b]
        parts = []
        for p in part:
            w = K[2 * p] - a
            s = 0
            while s < w:
                ws = min(SEG, w - s)
                align(1, 2 * ws)
                off = cur[1]
                parts.append((s, ws, 1, off))
                segs.append((2 * p, a + s, ws, 1, off, 0))
                segs.append((2 * p + 1, a + s, ws, 1, off + ws, 0))
                cur[1] += 2 * ws
                s += ws
        slabs = []
        for p in full:
            dt, path = SLAB_PATTERN[sli % len(SLAB_PATTERN)]
            sli += 1
            align(dt, slab)
            off = cur[dt]
            for si in range(n_s):
                o = off + si * 2 * ws_f
                segs.append((2 * p, a + si * ws_f, ws_f, dt, o, path))
                segs.append((2 * p + 1, a + si * ws_f, ws_f, dt, o + ws_f,
                             path))
            slabs.append((dt, off, path))
            cur[dt] += slab
        chunks.append((a, b, slabs, parts))
        a = b
    SH = [((c + DTILE - 1) // DTILE) * DTILE for c in cur]
    return chunks, segs, SH


def _build_l1():
    nc = bacc.Bacc("TRN2", debug=False, num_devices=CORES)
    xt65 = nc.dram_tensor("xt65", [66, NPAD1], bf16, kind="ExternalInput")
    wab = nc.dram_tensor("wab", [66, 128], bf16, kind="ExternalInput")
    ab = nc.dram_tensor("ab", [128, NPAD1], bf16, kind="ExternalOutput")

    QW = 2560  # 5 tiles of 512 per DMA piece
    with tile.TileContext(nc) as tc:
        with (
            tc.tile_pool(name="big", bufs=1) as big,
            tc.tile_pool(name="psum", bufs=4, space="PSUM") as psum,
        ):
            wt = big.tile([66, 128], bf16)
            xt = big.tile([66, NPAD1], bf16)
            abt = big.tile([128, NPAD1], bf16)
            nc.sync.dma_start(out=wt[:], in_=wab[:, :])
            for q in range(NPAD1 // QW):
                qs = slice(q * QW, (q + 1) * QW)
                nc.sync.dma_start(out=xt[:, qs], in_=xt65[:, qs])
            for c in range(NPAD1 // 512):
                sl = slice(c * 512, (c + 1) * 512)
                pt = psum.tile([128, 512], f32)
                nc.tensor.matmul(out=pt[:], lhsT=wt[:], rhs=xt[:, sl],
                                 start=True, stop=True)
                if c % 2 == 0:
                    nc.vector.tensor_copy(out=abt[:, sl], in_=pt[:])
                else:
                    nc.scalar.activation(
                        out=abt[:, sl], in_=pt[:],
                        func=mybir.ActivationFunctionType.Copy)
                if (c + 1) % 5 == 0:
                    qs = slice((c + 1) * 512 - QW, (c + 1) * 512)
                    nc.sync.dma_start(out=ab[:, qs], in_=abt[:, qs])
    nc.compile()
    return nc


def _build_l2(K):
    chunks, _segs, SH = _plane_schedule(K)
    nc = bacc.Bacc("TRN2", debug=False, num_devices=CORES)
    g16 = nc.dram_tensor("g16", [128, SH[0]], bf16, kind="ExternalInput")
    g8 = nc.dram_tensor("g8", [128, SH[1]], fp8, kind="ExternalInput")
    nb = nc.dram_tensor("nb", [128, NHALF], bf16, kind="ExternalInput")
    ic = nc.dram_tensor("ic", [128, NHALF], bf16, kind="ExternalInput")
    ini = nc.dram_tensor("ini", [128, NHALF], bf16, kind="ExternalInput")
    xu = nc.dram_tensor("xu", [128, NHALF], bf16, kind="ExternalInput")
    wu = nc.dram_tensor("wu", [128, 128], bf16, kind="ExternalInput")
    bu = nc.dram_tensor("bu", [64, 1], f32, kind="ExternalInput")
    upd = nc.dram_tensor("upd", [128, NHALF], bf16, kind="ExternalOutput")

    amax = mybir.AluOpType.max
    mult = mybir.AluOpType.mult
    gdram = (g16, g8)
    gdt = (bf16, fp8)
    ntile = (SH[0] // DTILE, SH[1] // DTILE)

    with tile.TileContext(nc) as tc:
        with (
            tc.tile_pool(name="persist", bufs=1) as per,
            tc.tile_pool(name="st16", bufs=3) as st16p,
            tc.tile_pool(name="st8", bufs=3) as st8p,
            tc.tile_pool(name="ybuf", bufs=4) as ybuf,
            tc.tile_pool(name="ypart", bufs=6) as ypart,
            tc.tile_pool(name="abuf", bufs=3) as abuf,
            tc.tile_pool(name="rbuf", bufs=4) as rbuf,
            tc.tile_pool(name="obuf", bufs=3) as obuf,
            tc.tile_pool(name="acc", bufs=2, space="PSUM") as accp,
            tc.tile_pool(name="upsum", bufs=2, space="PSUM") as upsum,
            tc.tile_pool(name="stage", bufs=2, space="PSUM") as stagep,
        ):
            nb_t = per.tile([128, NHALF], bf16)
            ic_t = per.tile([128, NHALF], bf16)
            ini_t = per.tile([128, NHALF], bf16)
            xu_t = per.tile([128, NHALF], bf16)
            wu_t = per.tile([128, 128], bf16)
            bu_t = per.tile([64, 1], f32)
            agg_t = per.tile([128, NHALF], bf16)
            out_t = per.tile([128, NHALF], bf16)
            nbp_t = per.tile([128, NHALF], bf16)
            ident = per.tile([128, 128], bf16)
            ident8 = per.tile([128, 128], fp8)
            nc.scalar.dma_start(out=wu_t[:], in_=wu[:, :])
            nc.scalar.dma_start(out=bu_t[:], in_=bu[:, :])
            make_identity(nc, ident[:])
            if HAS_RELU_PATH:
                make_identity(nc, ident8[:])

            st_tiles = [{}, {}]
            stp = (st16p, st8p)

            def stile(dt, i):
                cachebin = st_tiles[dt]
                if i not in cachebin:
                    t = stp[dt].tile([128, DTILE], gdt[dt], tag="st")
                    h = DTILE // 2
                    nc.sync.dma_start(
                        out=t[:, 0:h],
                        in_=gdram[dt][:, i * DTILE:i * DTILE + h])
                    nc.sync.dma_start(
                        out=t[:, h:],
                        in_=gdram[dt][:, i * DTILE + h:(i + 1) * DTILE])
                    cachebin[i] = t
                return cachebin[i]

            def prologue(ci):
                a, b, _s, _p = chunks[ci]
                nc.sync.dma_start(out=nb_t[:, a:b], in_=nb[:, a:b])
                nc.sync.dma_start(out=ini_t[:, a:b], in_=ini[:, a:b])
                if HAS_RELU_PATH:
                    nc.vector.tensor_scalar_mul(out=nbp_t[:, a:b],
                                                in0=nb_t[:, a:b],
                                                scalar1=-1.0)

            def finish(a, b, acc_t):
                # per-tile: SBUF copy of acc (ACT), agg = copy * (1/count)
                # (DVE), update MLP (PE), relu+bias (ACT) into chunk-wide
                # per-half buffers; one out DMA per half per chunk
                och0 = obuf.tile([64, CHUNKN], bf16, tag="ot0")
                och1 = obuf.tile([64, CHUNKN], bf16, tag="ot1")
                och = (och0, och1)
                t0 = a
                while t0 < b:
                    w = min(SEG, b - t0)
                    sl = slice(t0, t0 + w)
                    at = abuf.tile([128, SEG], bf16, tag="at")
                    nc.scalar.activation(
                        out=at[:, 0:w], in_=acc_t[:, t0 - a:t0 - a + w],
                        func=mybir.ActivationFunctionType.Copy)
                    nc.vector.tensor_tensor(out=agg_t[:, sl], in0=at[:, 0:w],
                                            in1=ic_t[:, sl], op=mult)
                    for h in range(2):
                        ps = slice(64 * h, 64 * h + 64)
                        pt = upsum.tile([64, SEG], f32, tag=f"pt{h}")
                        nc.tensor.matmul(out=pt[:, 0:w], lhsT=wu_t[ps, 0:64],
                                         rhs=xu_t[ps, sl],
                                         start=True, stop=False)
                        nc.tensor.matmul(out=pt[:, 0:w],
                                         lhsT=wu_t[ps, 64:128],
                                         rhs=agg_t[ps, sl],
                                         start=False, stop=True)
                        nc.scalar.activation(
                            out=och[h][:, t0 - a:t0 - a + w], in_=pt[:, 0:w],
                            func=mybir.ActivationFunctionType.Relu,
                            bias=bu_t[:])
                    t0 += w
                for h in range(2):
                    ps = slice(64 * h, 64 * h + 64)
                    q = (nc.sync, nc.scalar)[h]
                    q.dma_start(out=upd[ps, a:b], in_=och[h][:, 0:b - a])

            prologue(0)
            prev = None
            for ci, (a, b, slabs, parts) in enumerate(chunks):
                w_ch = b - a
                ws_f = min(SEG, w_ch)
                n_s = (w_ch + ws_f - 1) // ws_f
                slab = 2 * w_ch
                acc_t = accp.tile([128, CHUNKN], f32, tag="acc")
                # ISA: matmul rhs <= [128, 512]
                for s0 in range(0, w_ch, SEG):
                    w = min(SEG, w_ch - s0)
                    nc.tensor.matmul(out=acc_t[:, s0:s0 + w], lhsT=ident[:],
                                     rhs=ini_t[:, a + s0:a + s0 + w],
                                     start=True, stop=False)
                if ci + 1 < len(chunks):
                    prologue(ci + 1)
                n_inj = 2 * n_s * len(slabs) + 2 * len(parts)
                inj = 0
                pend = None

                def inject(rhs_ap, s0, ws, last):
                    nc.tensor.matmul(out=acc_t[:, s0:s0 + ws], lhsT=ident[:],
                                     rhs=rhs_ap, start=False, stop=last)

                for (s, ws, dt, off) in parts:
                    st = stile(dt, off // DTILE)
                    la = off % DTILE
                    cols = slice(a + s, a + s + ws)
                    yt = ypart.tile([128, 2 * SEG], bf16, tag="yp")
                    nbb = nb_t[:, cols].unsqueeze(1).to_broadcast([128, 2, ws])
                    nc.vector.tensor_tensor(out=yt[:, 0:2 * ws],
                                            in0=st[:, la:la + 2 * ws],
                                            in1=nbb, op=amax)
                    inj += 2
                    inject(yt[:, 0:ws], s, ws, False)
                    inject(yt[:, ws:2 * ws], s, ws, inj == n_inj)
                # -B' pattern matching one slab's [s x (A|B)] layout
                base = nb_t[:, a:b]
                if n_s > 1:
                    v = base.rearrange("p (s c) -> p s c", s=n_s)
                    v = v.unsqueeze(2).to_broadcast([128, n_s, 2, ws_f])
                else:
                    v = base.unsqueeze(1).to_broadcast([128, 2, w_ch])
                for si_, (dt, off, path) in enumerate(slabs):
                    ti = off // DTILE
                    la = off % DTILE
                    st = stile(dt, ti)
                    if ti + 1 < ntile[dt]:
                        stile(dt, ti + 1)  # prefetch
                    if si_ == 2 and prev is not None:
                        # software pipeline: previous chunk's finish behind
                        # this chunk's first slabs
                        finish(*prev)
                        prev = None
                    if path == 1:
                        # relu path: stage g + B' in PSUM (PE), relu on ACT,
                        # re-inject one unit behind so PE never waits on ACT.
                        for si in range(n_s):
                            for pl in range(2):
                                lo = la + si * 2 * ws_f + pl * ws_f
                                cs = slice(a + si * ws_f, a + (si + 1) * ws_f)
                                sg = stagep.tile([128, SEG], f32, tag="sg")
                                nc.tensor.matmul(
                                    out=sg[:, 0:ws_f], lhsT=ident8[:],
                                    rhs=st[:, lo:lo + ws_f],
                                    start=True, stop=False)
                                nc.tensor.matmul(
                                    out=sg[:, 0:ws_f], lhsT=ident[:],
                                    rhs=nbp_t[:, cs], start=False, stop=True)
                                yo = rbuf.tile([128, SEG], bf16, tag="yo")
                                nc.scalar.activation(
                                    out=yo[:, 0:ws_f], in_=sg[:, 0:ws_f],
                                    func=mybir.ActivationFunctionType.Relu)
                                if pend is not None:
                                    inj += 1
                                    inject(pend[0][:, 0:pend[2]], pend[1],
                                           pend[2], inj == n_inj)
                                pend = (yo, si * ws_f, ws_f)
                        continue
                    yt = ybuf.tile([128, 2 * CHUNKN], bf16, tag="yt")
                    nc.vector.tensor_tensor(out=yt[:, 0:slab],
                                            in0=st[:, la:la + slab],
                                            in1=v, op=amax)
                    for si in range(n_s):
                        lo = si * 2 * ws_f
                        inj += 2
                        inject(yt[:, lo:lo + ws_f], si * ws_f, ws_f, False)
                        inject(yt[:, lo + ws_f:lo + 2 * ws_f],
                               si * ws_f, ws_f, inj == n_inj)
                if pend is not None:
                    inj += 1
                    inject(pend[0][:, 0:pend[2]], pend[1], pend[2],
                           inj == n_inj)
                    pend = None
                if prev is not None:
                    finish(*prev)
                # update-phase inputs, consumed by finish() one chunk later
                nc.scalar.dma_start(out=ic_t[:, a:b], in_=ic[:, a:b])
                nc.scalar.dma_start(out=xu_t[:, a:b], in_=xu[:, a:b])
                prev = (a, b, acc_t)
            finish(*prev)
    nc.compile()
    return nc


def kernel(x, edge_index, W_msg, b_msg, W_upd, b_upd):
    x = np.asarray(x, dtype=np.float32)
    src = np.asarray(edge_index[0], dtype=np.int64)
    tgt = np.asarray(edge_index[1], dtype=np.int64)
    W_msg = np.asarray(W_msg, dtype=np.float32)
    b_msg = np.asarray(b_msg, dtype=np.float32)
    W_upd = np.asarray(W_upd, dtype=np.float32)
    b_upd = np.asarray(b_upd, dtype=np.float32)

    # ---------------- L1 ----------------
    if "l1" not in _cache:
        _cache["l1"] = _build_l1()
    wab = np.zeros((66, 128), dtype=np.float32)
    wab[:64, :64] = W_msg[:64]
    wab[:64, 64:] = -W_msg[64:]
    wab[64, 64:] = -b_msg
    wab = wab.astype(BF)
    xb = x.astype(BF)
    in1 = []
    for c in range(CORES):
        xt65 = np.zeros((66, NPAD1), dtype=BF)
        xt65[:64, :NPC] = xb[c * NPC:(c + 1) * NPC].T
        xt65[64, :] = np.float32(1.0)
        in1.append({"xt65": xt65, "wab": wab})
    res1 = run_bass_kernel_spmd(_cache["l1"], in1, list(range(CORES)))
    A_T = np.concatenate([np.asarray(r["ab"])[0:64, :NPC]
                          for r in res1.results], axis=1)
    negB_T = np.concatenate([np.asarray(r["ab"])[64:128, :NPC]
                             for r in res1.results], axis=1)

    # ---------------- host: plane schedule ----------------
    counts = np.bincount(tgt, minlength=N_NODES).astype(np.int64)
    order = np.argsort(tgt, kind="stable")
    cum = np.zeros(N_NODES + 1, dtype=np.int64)
    np.cumsum(counts, out=cum[1:])

    colloc = np.empty((CORES, 2, NHALF), dtype=np.int64)
    colnode = np.empty((CORES, 2, NHALF), dtype=np.int64)
    colcnt = np.zeros((CORES, 2, NHALF), dtype=np.int64)
    for c in range(CORES):
        lo = c * NPC
        cnt_loc = np.zeros(2 * NHALF, dtype=np.int64)
        cnt_loc[:NPC] = counts[lo:lo + NPC]
        rank = np.argsort(-cnt_loc, kind="stable")
        for h in range(2):
            nodes = rank[h::2]
            colloc[c, h] = nodes
            colnode[c, h] = np.minimum(nodes, NPC - 1) + lo
            colcnt[c, h] = cnt_loc[nodes]

    tmax = int(colcnt.max())
    tmax += tmax % 2
    K = np.zeros(tmax, dtype=np.int64)
    for j in range(tmax):
        K[j] = int((colcnt > j).sum(axis=2).max())
    for p in range(tmax // 2):
        K[2 * p + 1] = K[2 * p]
    K = [int(k) for k in K if k > 0]
    if len(K) % 2:
        K.append(K[-1])

    key = ("l2", tuple(K))
    if key not in _cache:
        _cache[key] = _build_l2(K)
    chunks, segs, SH = _plane_schedule(K)

    # cnt_corr: planes covering each column via the max-path only (the
    # relu path needs no correction; its pads contribute exactly zero)
    cnt_corr = np.zeros(NHALF, dtype=np.float32)
    for (_jj, col0, ws, _dt, _off, path) in segs:
        if path == 0:
            cnt_corr[col0:col0 + ws] += 1

    A_np = np.asarray(A_T)
    A8 = A_np.astype(F8)
    nB_np = np.asarray(negB_T)
    xbT = np.ascontiguousarray(xb.T)
    wu = np.zeros((128, 128), dtype=np.float32)
    wu[:64, :64] = W_upd[:64]
    wu[:64, 64:] = W_upd[64:]
    wu[64:] = wu[:64]
    wu = wu.astype(BF)
    bu = b_upd.reshape(64, 1).astype(np.float32)

    in2 = []
    for c in range(CORES):
        G16 = np.full((128, SH[0]), PAD_G, dtype=BF)
        G8 = np.full((128, SH[1]), PAD_G, dtype=F8)
        for h in range(2):
            nodes = colnode[c, h]
            ncnt = colcnt[c, h]
            starts = cum[nodes]
            for dt, Gt, At in ((0, G16, A_np), (1, G8, A8)):
                srcflat = np.full(SH[dt], -1, dtype=np.int64)
                for (jj, col0, ws, sdt, off, _path) in segs:
                    if sdt != dt:
                        continue
                    csl = slice(col0, col0 + ws)
                    valid = ncnt[csl] > jj
                    srcflat[off:off + ws] = np.where(
                        valid, starts[csl] + jj, -1)
                have = srcflat >= 0
                idx = src[order[srcflat[have]]]
                Gh = np.full((64, SH[dt]), PAD_G, dtype=Gt.dtype)
                Gh[:, have] = At[:, idx]
                Gt[64 * h:64 * h + 64] = Gh
        nb2 = np.empty((128, NHALF), dtype=BF)
        ic2 = np.empty((128, NHALF), dtype=BF)
        ini2 = np.empty((128, NHALF), dtype=BF)
        xu2 = np.empty((128, NHALF), dtype=BF)
        for h in range(2):
            r = slice(64 * h, 64 * h + 64)
            nbh = nB_np[:, colnode[c, h]]
            nb2[r] = nbh
            xu2[r] = xbT[:, colnode[c, h]]
            ic2[r] = (1.0 / np.maximum(colcnt[c, h], 1)).astype(BF)[None, :]
            # ini = cnt_corr * B' = (-cnt_corr) * (-B')
            ini2[r] = ((-cnt_corr[None, :]) *
                       nbh.astype(np.float32)).astype(BF)
        in2.append({"g16": G16, "g8": G8, "nb": nb2, "ic": ic2, "ini": ini2,
                    "xu": xu2, "wu": wu, "bu": bu})

    res2 = run_bass_kernel_spmd(_cache[key], in2, list(range(CORES)))

    out = np.empty((N_NODES, 64), dtype=np.float32)
    for c in range(CORES):
        upd = np.asarray(res2.results[c]["upd"]).astype(np.float32)
        lo = c * NPC
        for h in range(2):
            loc = colloc[c, h]
            real = loc < NPC
            vals = upd[64 * h:64 * h + 64, :].T
            out[lo + loc[real]] = vals[real]
    return out
